# revision 1
# baseline (speedup 1.0000x reference)
"""Trainium2 Bass kernel for nn_DetectorLoss (SIoU detector loss).

Strategy: data-parallel over batch N=16 -> 8 cores x 2 batches.
Per core: dma_gather pulls the per-candidate pred values (256B rows),
one-hot extraction, SIoU/cls math on DVE+ACT (Exp/Ln table set only),
dense obj baseline = 0.375*sum(pobj^2).  A host roundtrip between two
NEFFs carries the single global scalar (iou_mean); phase B applies the
f-mask, dedups duplicate-cell winners via a column-shift trick, and
reduces the loss partials.  Host combines per-core partial sums.
"""

import numpy as np

import concourse.bass as bass
import concourse.mybir as mybir
from concourse import library_config
from concourse.bass import AP
from concourse.library_overlay import lower_extended_insts
from concourse.tile import TileContext
from concourse.bass_utils import run_bass_kernel_spmd

# ---------------- problem constants (hardcoded per spec) ----------------
N, C, H, W = 16, 85, 160, 160
HW = H * W                  # 25600
CHW = C * HW                # 2176000
NCORES = 8
BPC = 2                     # batches per core
SHARD = BPC * CHW           # elems per core shard
M = 4096
NCAND = 4 * M

f32 = mybir.dt.float32
i16 = mybir.dt.int16
Alu = mybir.AluOpType
Act = mybir.ActivationFunctionType
X = mybir.AxisListType.X

MAX_WAITS = 1


def _split_excess_waits(nc):
    """This neuronxcc build rejects TPB_CTRL-class instructions (Drain/NoOp)
    with >1 sem wait; hoist extras onto same-engine Drains placed
    immediately before (Drains are never elided by codegen).  Compute/DMA
    instructions keep their multi-wait encoding."""
    ctrl = (mybir.InstDrain, mybir.InstNoOp, mybir.InstISA)
    for f in nc.m.functions:
        for bb in f.blocks:
            new_list = []
            for ins in bb.instructions:
                si = ins.sync_info
                if si is not None and len(si.on_wait) > MAX_WAITS:
                    waits = list(si.on_wait)
                    excess, keep = waits[:-MAX_WAITS], waits[-MAX_WAITS:]
                    while excess:
                        chunk, excess = excess[:MAX_WAITS], excess[MAX_WAITS:]
                        carrier = mybir.InstDrain(
                            name=nc.get_next_instruction_name(),
                            engine=ins.engine, ins=[], outs=[],
                            bass_is_fusable=False,
                            sync_info=mybir.SyncInfo(on_wait=chunk, on_update=[]),
                        )
                        nc.register_instruction(carrier)
                        new_list.append(carrier)
                    si.on_wait = keep
                new_list.append(ins)
            bb.instructions[:] = new_list


def _bc(ap, reps, dim):
    """Insert a stride-0 broadcast dim of size `reps` at free-dim index
    `dim` (0 = right after partition dim)."""
    pattern = list(ap.ap)
    pattern.insert(dim + 1, [0, reps])
    return AP(tensor=ap.tensor, offset=ap.offset, ap=pattern)


# ---------------- host preparation ----------------

def _prep(preds, targets):
    preds = np.asarray(preds, np.float32)
    targets = np.asarray(targets, np.float32)
    assert preds.shape == (N, C, H, W) and targets.shape[1] == 6

    dt = np.float32
    # build_target, mirroring reference.py exactly (f32/int ops are exact)
    scale = np.array([1, 1, W, H, W, H], dt)
    gt = (targets * scale).astype(dt)
    gt4 = np.broadcast_to(gt, (4, targets.shape[0], 6))
    quad = np.array([[0, 0], [1, 0], [0, 1], [1, 1]], np.int32)
    gij = gt4[..., 2:4].astype(np.int32) + quad[:, None, :]
    m = (np.min(np.where(gij < H, gij, 0), axis=-1) > 0).reshape(-1)
    gi = np.where(m, gij[..., 0].reshape(-1), 0)
    gj = np.where(m, gij[..., 1].reshape(-1), 0)
    b = np.tile(targets[:, 0].astype(np.int32), 4)
    gbox = gt4[..., 2:].reshape(-1, 4).astype(dt)
    gcls = np.tile(targets[:, 1].astype(np.int32), 4)
    cnt_m = max(int(m.sum()), 1)

    # box2-derived constants (f32, same rounding as reference)
    gx, gy, gw, gh = gbox[:, 0], gbox[:, 1], gbox[:, 2], gbox[:, 3]
    half = dt(0.5)
    b2x1 = (gx - gw * half).astype(dt)
    b2x2 = (gx + gw * half).astype(dt)
    b2y1 = (gy - gh * half).astype(dt)
    b2y2 = (gy + gh * half).astype(dt)
    w2 = (b2x2 - b2x1).astype(dt)
    h2 = ((b2y2 - b2y1) + dt(1e-7)).astype(dt)
    area2h = (w2 * h2).astype(dt)
    sx2 = (b2x1 + b2x2).astype(dt)
    sy2 = (b2y1 + b2y2).astype(dt)

    core = b >> 1
    bl = b & 1
    rr = gj * W + gi           # flat cell within a batch image
    row64 = rr >> 6
    phase = rr & 63

    # ---- per (core, batch) packing: dup-cell groups -> same row, adjacent cols
    placements = {}            # (k, lb) -> list of rows, each row = list of cand idx
    max_cols = 0
    for k in range(NCORES):
        for lb in range(BPC):
            cand = np.where((core == k) & (bl == lb))[0]   # ascending orig order
            groups = {}
            for ci in cand:
                if m[ci]:
                    groups.setdefault(rr[ci], []).append(ci)
            grouped = [v for v in groups.values() if len(v) > 1]
            in_group = set(x for v in grouped for x in v)
            singles = [ci for ci in cand if ci not in in_group]
            rows = [[] for _ in range(128)]
            order = sorted(range(128), key=lambda p: p)
            for gmem in sorted(grouped, key=len, reverse=True):
                p = min(range(128), key=lambda q: len(rows[q]))
                rows[p].extend(gmem)
            for ci in singles:
                p = min(range(128), key=lambda q: len(rows[q]))
                rows[p].append(ci)
            placements[(k, lb)] = rows
            max_cols = max(max_cols, max(len(r) for r in rows))

    GB = max_cols              # cols per batch block
    G = BPC * GB
    PB = 128 * GB
    KA, KB = 5 * PB // 16, PB // 16
    KTOT = BPC * (KA + KB)

    NF = 14                    # hostf fields
    per_core = []
    for k in range(NCORES):
        slot = -np.ones((128, G), np.int64)    # candidate index per slot
        grp = np.zeros((128, G), np.int64)     # group id (cell) for sibling masks
        for lb in range(BPC):
            rows = placements[(k, lb)]
            for p in range(128):
                for j, ci in enumerate(rows[p]):
                    slot[p, lb * GB + j] = ci
                    grp[p, lb * GB + j] = ((lb + 1) * (1 << 20) + int(rr[ci])) if m[ci] else 0

        filled = slot >= 0
        sidx = np.where(filled, slot, 0)

        hostf = np.zeros((128, NF, G), np.float32)
        hostf[:, 0] = np.where(filled, gi[sidx], 0).astype(dt)
        hostf[:, 1] = np.where(filled, gj[sidx], 0).astype(dt)
        hostf[:, 2] = np.where(filled, m[sidx], False).astype(dt)
        hostf[:, 3] = (filled & (np.arange(G)[None, :] < GB)).astype(dt)
        hostf[:, 4] = (filled & (np.arange(G)[None, :] >= GB)).astype(dt)
        hostf[:, 5] = np.where(filled, b2x1[sidx], 0.0)
        hostf[:, 6] = np.where(filled, b2y1[sidx], 0.0)
        hostf[:, 7] = np.where(filled, b2x2[sidx], 1.0)
        hostf[:, 8] = np.where(filled, b2y2[sidx], 1.0)
        hostf[:, 9] = np.where(filled, sx2[sidx], 1.0)
        hostf[:, 10] = np.where(filled, sy2[sidx], 1.0)
        hostf[:, 11] = np.where(filled, w2[sidx], 1.0)
        hostf[:, 12] = np.where(filled, h2[sidx], 1.0)
        hostf[:, 13] = np.where(filled, area2h[sidx], 1.0)

        # sibling masks: e1 -> next col is same dup-group, e2 -> col+2 is
        ggrp = grp * (grp > 0)
        e1 = np.zeros((128, G), np.float32)
        e2 = np.zeros((128, G), np.float32)
        e1[:, :-1] = ((ggrp[:, :-1] == ggrp[:, 1:]) & (ggrp[:, :-1] > 0)).astype(dt)
        e2[:, :-2] = ((ggrp[:, :-2] == ggrp[:, 2:]) & (ggrp[:, :-2] > 0)).astype(dt)

        # one-hot for extraction (zero for pad slots -> extracted value 0)
        oneh = np.zeros((128, G, 64), np.float32)
        pp, cc = np.where(filled)
        oneh[pp, cc, phase[slot[pp, cc]]] = 1.0

        # int16 gather row indices
        def wrap16(idxs):
            n = idxs.shape[0]
            base16 = idxs.reshape(n // 16, 16).T.astype(np.int16)   # [16, n/16]
            return np.tile(base16, (8, 1))                          # [128, n/16]

        idx16 = np.zeros((128, KTOT), np.int16)
        off = 0
        for lb in range(BPC):
            blk = slice(lb * GB, (lb + 1) * GB)
            r64 = np.where(filled[:, blk], row64[sidx[:, blk]], 0)  # [128, GB]
            # gather A: channels 0..4, idx j = ch*PB + cb*128 + p
            ja = np.empty((5, GB, 128), np.int64)
            for ch in range(5):
                ja[ch] = (ch * 400 + r64).T                          # [GB, 128]
            idx16[:, off:off + KA] = wrap16(ja.reshape(-1))
            off += KA
            # gather B: class channel, row = gcls*400 + r64
            cls_row = np.where(filled[:, blk], gcls[sidx[:, blk]] * 400
                               + row64[sidx[:, blk]], 0)
            idx16[:, off:off + KB] = wrap16(cls_row.T.reshape(-1))
            off += KB

        shard = np.ascontiguousarray(preds[BPC * k:BPC * (k + 1)]).reshape(-1)
        pobjd = np.ascontiguousarray(
            preds[BPC * k:BPC * (k + 1), 0]).reshape(128, 400)

        per_core.append(dict(
            shard=shard, pobjd=pobjd, idx16=idx16,
            hostf=hostf.reshape(128, NF * G), oneh=oneh.reshape(128, G * 64),
            hostf2=np.concatenate(
                [hostf[:, 2], hostf[:, 3], hostf[:, 4], e1, e2],
                axis=1).astype(np.float32),
        ))

    meta = dict(GB=GB, G=G, PB=PB, KA=KA, KB=KB, KTOT=KTOT, NF=NF,
                cnt_m=cnt_m)
    return per_core, meta


# ---------------- phase A program ----------------

def _build_phase_a(meta):
    GB, G, PB = meta["GB"], meta["G"], meta["PB"]
    KA, KB, KTOT, NF = meta["KA"], meta["KB"], meta["KTOT"], meta["NF"]
    AOUT = 3 * G + 4

    nc = bass.Bass("TRN2", debug=False, num_swdge_queues=4)
    shard = nc.dram_tensor("shard", [SHARD], f32, kind="ExternalInput")
    idx16 = nc.dram_tensor("idx16", [128, KTOT], i16, kind="ExternalInput")
    hostf = nc.dram_tensor("hostf", [128, NF * G], f32, kind="ExternalInput")
    oneh = nc.dram_tensor("oneh", [128, G * 64], f32, kind="ExternalInput")
    pobjd = nc.dram_tensor("pobjd", [128, 400], f32, kind="ExternalInput")
    aout = nc.dram_tensor("aout", [128, AOUT], f32, kind="ExternalOutput")

    with TileContext(nc) as tc:
        with tc.tile_pool(name="sbuf", bufs=1) as pool:
            nc.gpsimd.load_library(library_config.mlp)

            idx_t = pool.tile([128, KTOT], i16)
            nc.sync.dma_start(out=idx_t[:], in_=idx16.ap())
            hf = pool.tile([128, NF, G], f32)
            nc.sync.dma_start(
                out=hf[:], in_=hostf.ap().rearrange("p (f g) -> p f g", f=NF))
            oh = pool.tile([128, G, 64], f32)
            nc.sync.dma_start(
                out=oh[:], in_=oneh.ap().rearrange("p (g e) -> p g e", e=64))
            pod = pool.tile([128, 400], f32)
            nc.sync.dma_start(out=pod[:], in_=pobjd.ap())

            out_t = pool.tile([128, AOUT], f32)
            nc.vector.memset(out_t[:], 0.0)

            # warm the Exp/Ln ACT table set before the gathers finish
            warm = pool.tile([128, 1], f32)
            nc.vector.memset(warm[:], 1.0)
            nc.scalar.activation(warm[:], warm[:], Act.Exp)
            nc.scalar.activation(warm[:], warm[:], Act.Ln)

            def F(i):            # hostf field view [128, G]
                return hf[:, i, :]

            def F2(i):           # two adjacent fields as [128, 2, G]
                return hf[:, i:i + 2, :]

            # ---- gathers: 6 fields x 256B rows per candidate ----
            gall = []
            sap = shard.ap()
            for lb in range(BPC):
                g6 = pool.tile([128, 6 * GB, 64], f32, name=f"g6_{lb}", tag=f"g6_{lb}")
                base = lb * CHW
                inA = sap[base:base + 5 * HW].rearrange("(r e) -> r e", e=64)
                inB = sap[base + 5 * HW:base + CHW].rearrange(
                    "(r e) -> r e", e=64)
                o = lb * (KA + KB)
                nc.gpsimd.dma_gather(
                    out_ap=g6[:, 0:5 * GB, :], in_ap=inA,
                    idxs_ap=idx_t[:, o:o + KA],
                    num_idxs=5 * PB, num_idxs_reg=5 * PB, elem_size=64,
                    single_packet=False, queue_num=lb * 2)
                nc.gpsimd.dma_gather(
                    out_ap=g6[:, 5 * GB:6 * GB, :], in_ap=inB,
                    idxs_ap=idx_t[:, o + KA:o + KA + KB],
                    num_idxs=PB, num_idxs_reg=PB, elem_size=64,
                    single_packet=False, queue_num=lb * 2 + 1)
                gall.append(g6)

            # ---- extraction: multiply by one-hot, reduce the 64-lane ----
            ext = pool.tile([128, 6, G], f32)
            for lb in range(BPC):
                prod = pool.tile([128, 6, GB, 64], f32, name=f"prod{lb}", tag=f"prod{lb}")
                oh_b = oh[:, lb * GB:(lb + 1) * GB, :]
                nc.vector.tensor_tensor(
                    out=prod[:],
                    in0=gall[lb][:].rearrange("p (f c) e -> p f c e", f=6),
                    in1=_bc(oh_b, 6, 0),
                    op=Alu.mult)
                nc.vector.tensor_reduce(
                    out=ext[:, :, lb * GB:(lb + 1) * GB], in_=prod[:],
                    axis=X, op=Alu.add)

            epobj = ext[:, 0, :]
            epr01 = ext[:, 1:3, :]
            epr23 = ext[:, 3:5, :]
            ecls = ext[:, 5, :]

            def T2(tag):
                return pool.tile([128, 2, G], f32, name=tag, tag=tag)[:]

            def T1(tag):
                return pool.tile([128, G], f32, name=tag, tag=tag)[:]

            ts = nc.vector.tensor_scalar
            tt = nc.vector.tensor_tensor
            act = nc.scalar.activation

            # tanh(pr01) = 1 - 2/(exp(2x)+1)
            t01 = T2("t01")
            act(t01, epr01, Act.Exp, scale=2.0)
            ts(t01, t01, 1.0, None, Alu.add)
            nc.vector.reciprocal(t01, t01)
            ts(t01, t01, -2.0, 1.0, Alu.mult, Alu.add)
            # pwh2 = 80*sigmoid(pr23) = 80/(1+exp(-x))
            pwh2 = T2("pwh2")
            act(pwh2, epr23, Act.Exp, scale=-1.0)
            ts(pwh2, pwh2, 1.0, None, Alu.add)
            nc.vector.reciprocal(pwh2, pwh2)
            ts(pwh2, pwh2, 80.0, None, Alu.mult)

            gij_f = hf[:, 0:2, :]
            txy = T2("txy")
            tt(txy, t01, gij_f, Alu.add)
            b1a = T2("b1a")
            tt(b1a, txy, pwh2, Alu.subtract)
            b1b = T2("b1b")
            tt(b1b, txy, pwh2, Alu.add)

            wh1 = T2("wh1")
            tt(wh1, b1b, b1a, Alu.subtract)
            ts(wh1[:, 1, :], wh1[:, 1, :], 1e-7, None, Alu.add)  # h1 += eps

            area1 = T1("area1")
            tt(area1, wh1[:, 0, :], wh1[:, 1, :], Alu.mult)

            b2a = F2(5)       # (b2x1, b2y1)
            b2b = F2(7)       # (b2x2, b2y2)
            mn = T2("mn")
            tt(mn, b1b, b2b, Alu.min)
            mx = T2("mx")
            tt(mx, b1a, b2a, Alu.max)
            dcl = T2("dcl")
            tt(dcl, mn, mx, Alu.subtract)
            ts(dcl, dcl, 0.0, None, Alu.max)
            inter = T1("inter")
            tt(inter, dcl[:, 0, :], dcl[:, 1, :], Alu.mult)

            u = T1("u")
            tt(u, area1, F(13), Alu.add)
            tt(u, u, inter, Alu.subtract)
            ts(u, u, 1e-7, None, Alu.add)
            invu = T1("invu")
            nc.vector.reciprocal(invu, u)
            iou0 = T1("iou0")
            tt(iou0, inter, invu, Alu.mult)

            cwh = T2("cwh")
            mx2 = T2("mx2")
            tt(mx2, b1b, b2b, Alu.max)
            mn2 = T2("mn2")
            tt(mn2, b1a, b2a, Alu.min)
            tt(cwh, mx2, mn2, Alu.subtract)

            scw = T2("scw")
            tt(scw, F2(9), b1a, Alu.subtract)       # (sx2,sy2) - b1x1y1
            tt(scw, scw, b1b, Alu.subtract)
            ts(scw, scw, 0.5, None, Alu.mult)

            sq = T2("sq")
            tt(sq, scw, scw, Alu.mult)
            ssum = T1("ssum")
            tt(ssum, sq[:, 0, :], sq[:, 1, :], Alu.add)
            invsig = T1("invsig")
            act(invsig, ssum, Act.Ln)
            act(invsig, invsig, Act.Exp, scale=-0.5)   # rsqrt via exp/ln

            negs = T2("negs")
            ts(negs, scw, -1.0, None, Alu.mult)
            sabs = T2("sabs")
            tt(sabs, scw, negs, Alu.max)
            sin1 = T1("sin1")
            tt(sin1, sabs[:, 0, :], invsig, Alu.mult)
            sin2 = T1("sin2")
            tt(sin2, sabs[:, 1, :], invsig, Alu.mult)

            thr = float(np.float32(2 ** 0.5 / 2))
            thr_t = pool.tile([128, 1], f32, name="thr_t")
            nc.vector.memset(thr_t[:], thr)
            cgt = T1("cgt")
            tt(cgt, sin1, thr_t[:].to_broadcast([128, G]), Alu.is_gt)
            dsin = T1("dsin")
            tt(dsin, sin2, sin1, Alu.subtract)
            tt(dsin, cgt, dsin, Alu.mult)
            sina = T1("sina")
            tt(sina, sin1, dsin, Alu.add)

            # angle_cost = 2*sina*sqrt(1-sina^2); gamma = angle_cost-2
            sa2 = T1("sa2")
            tt(sa2, sina, sina, Alu.mult)
            om = T1("om")
            ts(om, sa2, -1.0, 1.0, Alu.mult, Alu.add)
            rt = T1("rt")
            act(rt, om, Act.Ln)
            act(rt, rt, Act.Exp, scale=0.5)            # sqrt via exp/ln
            gam = T1("gam")
            tt(gam, sina, rt, Alu.mult)
            ts(gam, gam, 2.0, -2.0, Alu.mult, Alu.add)

            invcw = T2("invcw")
            nc.vector.reciprocal(invcw, cwh)
            rho = T2("rho")
            tt(rho, scw, invcw, Alu.mult)
            tt(rho, rho, rho, Alu.mult)
            gr = T2("gr")
            tt(gr[:, 0, :], gam, rho[:, 0, :], Alu.mult)
            tt(gr[:, 1, :], gam, rho[:, 1, :], Alu.mult)
            act(gr, gr, Act.Exp)
            dist = T1("dist")
            tt(dist, gr[:, 0, :], gr[:, 1, :], Alu.add)
            ts(dist, dist, -1.0, 2.0, Alu.mult, Alu.add)

            wh2t = F2(11)
            dwh = T2("dwh")
            tt(dwh, wh1, wh2t, Alu.subtract)
            ts(negs, dwh, -1.0, None, Alu.mult)
            tt(dwh, dwh, negs, Alu.max)
            mxw = T2("mxw")
            tt(mxw, wh1, wh2t, Alu.max)
            nc.vector.reciprocal(mxw, mxw)
            omg = T2("omg")
            tt(omg, dwh, mxw, Alu.mult)
            act(omg, omg, Act.Exp, scale=-1.0)
            ts(omg, omg, -1.0, 1.0, Alu.mult, Alu.add)   # 1-exp(-omiga)
            tt(omg, omg, omg, Alu.mult)                  # ^2
            tt(omg, omg, omg, Alu.mult)                  # ^4
            shp = T1("shp")
            tt(shp, omg[:, 0, :], omg[:, 1, :], Alu.add)

            dsh = T1("dsh")
            tt(dsh, dist, shp, Alu.add)
            ts(dsh, dsh, -0.5, None, Alu.mult)
            iou_v = out_t[:, 0:G]
            tt(iou_v, iou0, dsh, Alu.add)

            # sum(iou*m) partial per partition
            scr = T1("scr")
            tt(scr, iou_v, F(2), Alu.mult)
            nc.vector.tensor_reduce(out=out_t[:, 3 * G:3 * G + 1], in_=scr,
                                    axis=X, op=Alu.add)

            # lnp
            pg = T1("pg")
            ts(pg, ecls, 1e-38, None, Alu.max)
            act(out_t[:, G:2 * G], pg, Act.Ln)

            # pobj at candidate cells, for phase B
            nc.vector.tensor_copy(out=out_t[:, 2 * G:3 * G], in_=epobj)

            # dense obj baseline partial: sum(pobj^2) per partition
            scr4 = pool.tile([128, 400], f32)
            nc.vector.tensor_tensor(out=scr4[:], in0=pod[:], in1=pod[:],
                                    op=Alu.mult)
            nc.vector.tensor_reduce(out=out_t[:, 3 * G + 1:3 * G + 2],
                                    in_=scr4[:], axis=X, op=Alu.add)

            nc.sync.dma_start(out=aout.ap(), in_=out_t[:])

    lower_extended_insts(nc)
    _split_excess_waits(nc)
    return nc


# ---------------- phase B program ----------------

def _build_phase_b(meta):
    G = meta["G"]
    AOUT = 3 * G + 4

    nc = bass.Bass("TRN2", debug=False)
    bin_ = nc.dram_tensor("bin", [128, AOUT], f32, kind="ExternalInput")
    hostf2 = nc.dram_tensor("hostf2", [128, 5 * G], f32, kind="ExternalInput")
    imean = nc.dram_tensor("imean", [128, 1], f32, kind="ExternalInput")
    bout = nc.dram_tensor("bout", [128, 8], f32, kind="ExternalOutput")

    with TileContext(nc) as tc:
        with tc.tile_pool(name="sbuf", bufs=1) as pool:
            nc.gpsimd.load_library(library_config.mlp)

            bi = pool.tile([128, AOUT], f32)
            nc.sync.dma_start(out=bi[:], in_=bin_.ap())
            h2 = pool.tile([128, 5, G], f32)
            nc.sync.dma_start(
                out=h2[:], in_=hostf2.ap().rearrange("p (f g) -> p f g", f=5))
            im = pool.tile([128, 1], f32)
            nc.sync.dma_start(out=im[:], in_=imean.ap())
            ob = pool.tile([128, 8], f32)
            nc.vector.memset(ob[:], 0.0)

            iou_v = bi[:, 0:G]
            lnp_v = bi[:, G:2 * G]
            pox = bi[:, 2 * G:3 * G]
            m_v, mk0, mk1 = h2[:, 0, :], h2[:, 1, :], h2[:, 2, :]
            e1_v, e2_v = h2[:, 3, :], h2[:, 4, :]

            ts = nc.vector.tensor_scalar
            tt = nc.vector.tensor_tensor

            fpad = pool.tile([128, G + 2], f32)
            nc.vector.memset(fpad[:], 0.0)
            f_v = fpad[:, 0:G]
            tt(f_v, iou_v, im[:, 0:1].to_broadcast([128, G]), Alu.is_gt)
            tt(f_v, f_v, m_v, Alu.mult)

            # winner mask: W = f * (1 - e1*f[:,c+1]) * (1 - e2*f[:,c+2])
            t1 = pool.tile([128, G], f32, name="t1", tag="t1")[:]
            tt(t1, e1_v, fpad[:, 1:G + 1], Alu.mult)
            ts(t1, t1, -1.0, 1.0, Alu.mult, Alu.add)
            t2 = pool.tile([128, G], f32, name="t2", tag="t2")[:]
            tt(t2, e2_v, fpad[:, 2:G + 2], Alu.mult)
            ts(t2, t2, -1.0, 1.0, Alu.mult, Alu.add)
            W_v = pool.tile([128, G], f32, name="W", tag="W")[:]
            tt(W_v, f_v, t1, Alu.mult)
            tt(W_v, W_v, t2, Alu.mult)

            # nperb (both batches) -> all partitions
            fm = pool.tile([128, 2, G], f32)
            tt(fm[:, 0, :], f_v, mk0, Alu.mult)
            tt(fm[:, 1, :], f_v, mk1, Alu.mult)
            np2 = pool.tile([128, 2], f32)
            nc.vector.tensor_reduce(out=np2[:], in_=fm[:], axis=X, op=Alu.add)
            npa = pool.tile([128, 2], f32)
            import concourse.bass_isa as bass_isa
            nc.gpsimd.partition_all_reduce(
                npa[:], np2[:], channels=128,
                reduce_op=bass_isa.ReduceOp.add)
            ts(npa[:], npa[:], 0.5, None, Alu.max)
            nc.vector.tensor_copy(out=ob[:, 4:6], in_=npa[:])
            inv = pool.tile([128, 2], f32)
            nc.vector.reciprocal(inv[:], npa[:])

            fv = pool.tile([128, G], f32, name="fv", tag="fv")[:]
            t3 = pool.tile([128, G], f32, name="t3", tag="t3")[:]
            ts(t3, mk0, inv[:, 0:1], None, Alu.mult)
            ts(fv, mk1, inv[:, 1:2], None, Alu.mult)
            tt(fv, fv, t3, Alu.add)
            ts(fv, fv, 6400.0, None, Alu.mult)

            # obj correction: W*(sl1(pobj-iou)*fval - 0.375*pobj^2)
            d = pool.tile([128, G], f32, name="d", tag="d")[:]
            tt(d, pox, iou_v, Alu.subtract)
            ad = pool.tile([128, G], f32, name="ad", tag="ad")[:]
            ts(ad, d, -1.0, None, Alu.mult)
            tt(ad, d, ad, Alu.max)
            one_t = pool.tile([128, 1], f32, name="one_t")
            nc.vector.memset(one_t[:], 1.0)
            cc = pool.tile([128, G], f32, name="cc", tag="cc")[:]
            tt(cc, ad, one_t[:].to_broadcast([128, G]), Alu.is_lt)
            q = pool.tile([128, G], f32, name="q", tag="q")[:]
            tt(q, d, d, Alu.mult)
            ts(q, q, 0.5, None, Alu.mult)
            l_ = pool.tile([128, G], f32, name="l_", tag="l_")[:]
            ts(l_, ad, 0.5, None, Alu.subtract)
            tt(q, q, l_, Alu.subtract)
            tt(q, cc, q, Alu.mult)
            tt(q, l_, q, Alu.add)          # q = sl1
            tt(q, q, fv, Alu.mult)
            po2 = pool.tile([128, G], f32, name="po2", tag="po2")[:]
            tt(po2, pox, pox, Alu.mult)
            ts(po2, po2, 0.375, None, Alu.mult)
            tt(q, q, po2, Alu.subtract)
            scr = pool.tile([128, G], f32, name="scr", tag="scr")[:]
            tt(scr, W_v, q, Alu.mult)
            nc.vector.tensor_reduce(out=ob[:, 3:4], in_=scr, axis=X, op=Alu.add)

            # S1 = sum f*(1-iou); S2 = sum f*lnp; cntf = sum f
            onem = pool.tile([128, G], f32, name="onem", tag="onem")[:]
            ts(onem, iou_v, -1.0, 1.0, Alu.mult, Alu.add)
            s1t = pool.tile([128, G], f32, name="s1t", tag="s1t")[:]
            tt(s1t, f_v, onem, Alu.mult)
            nc.vector.tensor_reduce(out=ob[:, 0:1], in_=s1t, axis=X, op=Alu.add)
            s2t = pool.tile([128, G], f32, name="s2t", tag="s2t")[:]
            tt(s2t, f_v, lnp_v, Alu.mult)
            nc.vector.tensor_reduce(out=ob[:, 1:2], in_=s2t, axis=X, op=Alu.add)
            nc.vector.tensor_reduce(out=ob[:, 2:3], in_=f_v, axis=X,
                                    op=Alu.add)

            nc.sync.dma_start(out=bout.ap(), in_=ob[:])

    lower_extended_insts(nc)
    _split_excess_waits(nc)
    return nc


# ---------------- main entry ----------------

_CACHE = {}


def kernel(preds, targets):
    per_core, meta = _prep(preds, targets)

    key = (meta["GB"],)
    if key not in _CACHE:
        _CACHE[key] = (_build_phase_a(meta), _build_phase_b(meta))
    nc_a, nc_b = _CACHE[key]

    core_ids = list(range(NCORES))
    in_maps_a = [dict(shard=d["shard"], idx16=d["idx16"], hostf=d["hostf"],
                      oneh=d["oneh"], pobjd=d["pobjd"]) for d in per_core]
    res_a = run_bass_kernel_spmd(nc_a, in_maps_a, core_ids)

    G = meta["G"]
    aouts = [res_a.results[k]["aout"] for k in core_ids]
    sum_im = sum(float(a[:, 3 * G].sum(dtype=np.float64)) for a in aouts)
    base = sum(float(a[:, 3 * G + 1].sum(dtype=np.float64)) for a in aouts)
    iou_mean = np.float32(sum_im) / np.float32(meta["cnt_m"])

    imean_arr = np.full((128, 1), iou_mean, np.float32)
    in_maps_b = [dict(bin=aouts[k], hostf2=per_core[k]["hostf2"],
                      imean=imean_arr) for k in core_ids]
    res_b = run_bass_kernel_spmd(nc_b, in_maps_b, core_ids)

    bouts = [res_b.results[k]["bout"] for k in core_ids]
    S1 = sum(float(o[:, 0].sum(dtype=np.float64)) for o in bouts)
    S2 = sum(float(o[:, 1].sum(dtype=np.float64)) for o in bouts)
    cnt_f = max(sum(float(o[:, 2].sum(dtype=np.float64)) for o in bouts), 1.0)
    corr = sum(float(o[:, 3].sum(dtype=np.float64)) for o in bouts)

    iou_loss = np.float32(S1 / cnt_f)
    cls_loss = np.float32(-S2 / cnt_f)
    obj_loss = np.float32((0.375 * base + corr) / (N * HW))
    loss = np.float32(iou_loss * 8 + obj_loss * 16 + cls_loss)
    return (iou_loss, obj_loss, cls_loss, loss)



# revision 3
# speedup vs baseline: 1.8896x; 1.8896x over previous
"""Trainium2 Bass kernel for nn_DetectorLoss (SIoU detector loss).

Strategy: data-parallel over batch N=16 -> 8 cores x 2 batches.

Host re-lays preds (input-independent permutations only):
  - regarr: per cell r a 16-float record [ch0..4 @ r | pad | ch0..4 @ r+160 | pad]
    so ONE 256B-aligned dma_gather descriptor pair covers all 4 quadrant
    candidates' obj+reg channels of a ground truth (window of 30 floats at
    16*r0, phase in {0,16,32,48} -> 4-wide one-hot extraction).
  - clsarr: plain [80, HW] class channels per batch; one 64-float row per
    (GT, y-row) covers both x cells; 64-wide one-hot extraction.

Phase A computes per-candidate SIoU iou, log-class prob, pobj and the
partial sum(iou*m); host combines the global iou_mean; phase B applies the
f-mask, computes the masked reductions and the dense obj baseline.
Cell-collision dedup (rare) and phi=63 class-row crossings (rare) are
patched exactly on host from the per-candidate outputs.
"""

import math
import numpy as np

import concourse.bass as bass
import concourse.mybir as mybir
from concourse import library_config
from concourse.bass import AP
from concourse.library_overlay import lower_extended_insts
from concourse.tile import TileContext
from concourse.bass_utils import run_bass_kernel_spmd

# ---------------- problem constants (hardcoded per spec) ----------------
N, C, H, W = 16, 85, 160, 160
HW = H * W                  # 25600
NCORES = 8
BPC = 2
M_DEFAULT = 4096

f32 = mybir.dt.float32
i16 = mybir.dt.int16
Alu = mybir.AluOpType
Act = mybir.ActivationFunctionType
X = mybir.AxisListType.X

REGROWS = BPC * HW * 16 // 64      # 12800
CLSROWS = 80 * HW // 64            # 32000 per batch

# hostf field indices
F_GIJ = 0      # 2
F_B2A = 2      # 2
F_B2B = 4      # 2
F_SXY = 6      # 2
F_WH2 = 8      # 2
F_AREA2 = 10
F_M = 11
F_MCLSV = 12
NF = 13

MAX_WAITS = 1


def _split_excess_waits(nc):
    """This neuronxcc build rejects >1 sem wait on several instruction
    classes; hoist extras onto same-engine Drain carriers placed before."""
    for f in nc.m.functions:
        for bb in f.blocks:
            new_list = []
            for ins in bb.instructions:
                si = ins.sync_info
                if si is not None and len(si.on_wait) > MAX_WAITS:
                    waits = list(si.on_wait)
                    excess, keep = waits[:-MAX_WAITS], waits[-MAX_WAITS:]
                    while excess:
                        chunk, excess = excess[:MAX_WAITS], excess[MAX_WAITS:]
                        carrier = mybir.InstDrain(
                            name=nc.get_next_instruction_name(),
                            engine=ins.engine, ins=[], outs=[],
                            bass_is_fusable=False,
                            sync_info=mybir.SyncInfo(on_wait=chunk, on_update=[]),
                        )
                        nc.register_instruction(carrier)
                        new_list.append(carrier)
                    si.on_wait = keep
                new_list.append(ins)
            bb.instructions[:] = new_list


def _V(tap, dims, extra_off=0):
    """Custom free-dim view of a tile AP (keeps the partition dim)."""
    return AP(tensor=tap.tensor, offset=tap.offset + extra_off,
              ap=[list(tap.ap[0])] + [list(d) for d in dims])


def _wrap16(idxs):
    n = idxs.shape[0]
    base16 = idxs.reshape(n // 16, 16).T.astype(np.int16)
    return np.tile(base16, (8, 1))


# ---------------- host preparation ----------------

def _prep(preds, targets):
    preds = np.asarray(preds, np.float32)
    targets = np.asarray(targets, np.float32)
    M = targets.shape[0]
    dt = np.float32

    scale = np.array([1, 1, W, H, W, H], dt)
    gt = (targets * scale).astype(dt)
    x0 = gt[:, 2].astype(np.int32)
    y0 = gt[:, 3].astype(np.int32)
    quad = np.array([[0, 0], [1, 0], [0, 1], [1, 1]], np.int32)
    gijx = x0[None, :] + quad[:, 0:1]
    gijy = y0[None, :] + quad[:, 1:2]
    m4 = (np.minimum(np.where(gijx < H, gijx, 0),
                     np.where(gijy < H, gijy, 0)) > 0)      # [4, M]
    b = targets[:, 0].astype(np.int32)
    gcls = targets[:, 1].astype(np.int32)

    gx, gy, gw, gh = gt[:, 2], gt[:, 3], gt[:, 4], gt[:, 5]
    half = dt(0.5)
    b2x1 = (gx - gw * half).astype(dt)
    b2x2 = (gx + gw * half).astype(dt)
    b2y1 = (gy - gh * half).astype(dt)
    b2y2 = (gy + gh * half).astype(dt)
    w2 = (b2x2 - b2x1).astype(dt)
    h2 = ((b2y2 - b2y1) + dt(1e-7)).astype(dt)
    area2h = (w2 * h2).astype(dt)
    sx2 = (b2x1 + b2x2).astype(dt)
    sy2 = (b2y1 + b2y2).astype(dt)

    cnt_m = max(int(m4.sum()), 1)
    r0 = (y0.astype(np.int64) * W + x0)
    core = b >> 1
    lbv_all = b & 1

    cnts = np.zeros((NCORES, 2), np.int64)
    for k in range(NCORES):
        cnts[k, 0] = int(((core == k) & (lbv_all == 0)).sum())
        cnts[k, 1] = int(((core == k) & (lbv_all == 1)).sum())
    J0 = int(max(1, math.ceil(cnts[:, 0].max() / 128)))
    J1 = int(max(1, math.ceil(cnts[:, 1].max() / 128)))
    Jr = J0 + J1
    G2 = 4 * Jr
    J2 = 2 * Jr

    per_core = []
    for k in range(NCORES):
        pc = preds[BPC * k:BPC * (k + 1)]
        reg = np.zeros((BPC, HW, 16), dt)
        t5 = pc[:, 0:5].reshape(BPC, 5, HW).transpose(0, 2, 1)
        reg[:, :, 0:5] = t5
        reg[:, :-W, 8:13] = t5[:, W:, :]
        clsarr = np.ascontiguousarray(pc[:, 5:85]).reshape(-1)
        pobjd = np.ascontiguousarray(pc[:, 0]).reshape(128, 400)

        hostf = np.zeros((128, NF, G2), dt)
        hostf[:, F_B2B:F_B2B + 2] = 1.0
        hostf[:, F_SXY:F_SXY + 2] = 1.0
        hostf[:, F_WH2:F_WH2 + 2] = 1.0
        hostf[:, F_AREA2] = 1.0
        oh4 = np.zeros((128, Jr, 4), dt)
        phic = np.full((128, J2), -1.0, dt)
        regg = np.zeros((2 * Jr, 128), np.int64)
        clsg0 = np.zeros((2 * J0, 128), np.int64)
        clsg1 = np.zeros((2 * J1, 128), np.int64)
        candcell = np.full((128, G2), -1, np.int64)
        candorig = np.full((128, G2), -1, np.int64)
        hostb = np.zeros((128, 3, G2), dt)
        crossing = []

        for lbv in (0, 1):
            gl = np.where((core == k) & (lbv_all == lbv))[0]
            joff = 0 if lbv == 0 else J0
            cg = clsg0 if lbv == 0 else clsg1
            for i, g in enumerate(gl):
                p = i % 128
                jrel = i // 128
                j = jrel + joff
                rr = int(r0[g])
                s = rr & 3
                bb0 = lbv * 6400 + (rr >> 2)
                regg[2 * j, p] = bb0
                regg[2 * j + 1, p] = min(bb0 + 1, REGROWS - 1)
                oh4[p, j, s] = 1.0
                for win in (0, 1):
                    yy = int(y0[g]) + win
                    if yy <= H - 1:
                        flat = int(gcls[g]) * HW + yy * W + int(x0[g])
                        cg[jrel * 2 + win, p] = flat >> 6
                        phic[p, j * 2 + win] = dt(flat & 63)
                for cell in (0, 1):
                    for win in (0, 1):
                        cw = cell * 2 + win
                        col = cw * Jr + j
                        q = win * 2 + cell
                        mm = bool(m4[q, g])
                        gi = (int(x0[g]) + cell) if mm else 0
                        gj = (int(y0[g]) + win) if mm else 0
                        hostf[p, F_GIJ + 0, col] = gi
                        hostf[p, F_GIJ + 1, col] = gj
                        hostf[p, F_M, col] = 1.0 if mm else 0.0
                        hostf[p, F_B2A + 0, col] = b2x1[g]
                        hostf[p, F_B2A + 1, col] = b2y1[g]
                        hostf[p, F_B2B + 0, col] = b2x2[g]
                        hostf[p, F_B2B + 1, col] = b2y2[g]
                        hostf[p, F_SXY + 0, col] = sx2[g]
                        hostf[p, F_SXY + 1, col] = sy2[g]
                        hostf[p, F_WH2 + 0, col] = w2[g]
                        hostf[p, F_WH2 + 1, col] = h2[g]
                        hostf[p, F_AREA2, col] = area2h[g]
                        hostf[p, F_MCLSV, col] = 1.0 if mm else 0.0
                        hostb[p, 0, col] = 1.0 if mm else 0.0
                        hostb[p, 1, col] = 1.0 - lbv
                        hostb[p, 2, col] = float(lbv)
                        candorig[p, col] = q * M + g
                        if mm:
                            candcell[p, col] = (int(b[g]) * HW + gj * W + gi)
                            if cell == 1:
                                yy = int(y0[g]) + win
                                flat = (int(gcls[g]) * HW + yy * W
                                        + int(x0[g]))
                                if (flat & 63) == 63:
                                    hostf[p, F_MCLSV, col] = 0.0
                                    pv = float(preds[BPC * k + lbv,
                                               5 + int(gcls[g]), yy,
                                               int(x0[g]) + 1])
                                    crossing.append((p, col, pv))

        idx16 = np.concatenate([
            _wrap16(regg.reshape(-1)),
            _wrap16(clsg0.reshape(-1)),
            _wrap16(clsg1.reshape(-1)),
        ], axis=1)

        per_core.append(dict(
            regarr=reg.reshape(-1), clsarr=clsarr, pobjd=pobjd,
            idx16=idx16, hostf=hostf.reshape(128, NF * G2),
            oh4=oh4.reshape(128, Jr * 4), phic=phic,
            hostb=hostb.reshape(128, 3 * G2),
            candcell=candcell, candorig=candorig, crossing=crossing,
        ))

    iota = np.broadcast_to(np.arange(64, dtype=dt), (128, 64)).copy()
    meta = dict(J0=J0, J1=J1, Jr=Jr, G2=G2, J2=J2, cnt_m=cnt_m, M=M,
                iota=iota)
    return per_core, meta


# ---------------- phase A program ----------------

def _build_phase_a(meta):
    J0, J1 = meta["J0"], meta["J1"]
    Jr, G2, J2 = meta["Jr"], meta["G2"], meta["J2"]
    KR = 2 * Jr * 128
    K0 = 2 * J0 * 128
    K1 = 2 * J1 * 128
    KTW = (KR + K0 + K1) // 16
    AOUT = 3 * G2 + 8

    nc = bass.Bass("TRN2", debug=False, num_swdge_queues=4)
    regT = nc.dram_tensor("regarr", [BPC * HW * 16], f32, kind="ExternalInput")
    clsT = nc.dram_tensor("clsarr", [BPC * 80 * HW], f32, kind="ExternalInput")
    idxT = nc.dram_tensor("idx16", [128, KTW], i16, kind="ExternalInput")
    hfT = nc.dram_tensor("hostf", [128, NF * G2], f32, kind="ExternalInput")
    oh4T = nc.dram_tensor("oh4", [128, Jr * 4], f32, kind="ExternalInput")
    phT = nc.dram_tensor("phic", [128, J2], f32, kind="ExternalInput")
    ioT = nc.dram_tensor("iota", [128, 64], f32, kind="ExternalInput")
    aoutT = nc.dram_tensor("aout", [128, AOUT], f32, kind="ExternalOutput")

    with TileContext(nc) as tc:
        with tc.tile_pool(name="sbuf", bufs=1) as pool:
            nc.gpsimd.load_library(library_config.mlp)

            idx_t = pool.tile([128, KTW], i16)
            nc.sync.dma_start(out=idx_t[:], in_=idxT.ap())
            hf = pool.tile([128, NF, G2], f32)
            nc.sync.dma_start(
                out=hf[:], in_=hfT.ap().rearrange("p (f g) -> p f g", f=NF))
            oh4t = pool.tile([128, Jr, 4], f32)
            nc.sync.dma_start(
                out=oh4t[:], in_=oh4T.ap().rearrange("p (a b) -> p a b", b=4))
            pht = pool.tile([128, J2], f32)
            nc.sync.dma_start(out=pht[:], in_=phT.ap())
            iot = pool.tile([128, 64], f32)
            nc.sync.dma_start(out=iot[:], in_=ioT.ap())

            out_t = pool.tile([128, AOUT], f32)
            nc.vector.memset(out_t[:], 0.0)

            # ---- gathers ----
            gt_reg = pool.tile([128, 2 * Jr, 64], f32)
            nc.gpsimd.dma_gather(
                out_ap=gt_reg[:],
                in_ap=regT.ap().rearrange("(r e) -> r e", e=64),
                idxs_ap=idx_t[:, 0:KR // 16],
                num_idxs=KR, num_idxs_reg=KR, elem_size=64,
                single_packet=False, queue_num=0)
            gt_cls = pool.tile([128, J2 * 64 + 4], f32)
            nc.gpsimd.dma_gather(
                out_ap=gt_cls[:, 0:2 * J0 * 64].rearrange(
                    "p (a b) -> p a b", b=64),
                in_ap=clsT.ap()[0:80 * HW].rearrange("(r e) -> r e", e=64),
                idxs_ap=idx_t[:, KR // 16:(KR + K0) // 16],
                num_idxs=K0, num_idxs_reg=K0, elem_size=64,
                single_packet=False, queue_num=1)
            nc.gpsimd.dma_gather(
                out_ap=gt_cls[:, 2 * J0 * 64:J2 * 64].rearrange(
                    "p (a b) -> p a b", b=64),
                in_ap=clsT.ap()[80 * HW:].rearrange("(r e) -> r e", e=64),
                idxs_ap=idx_t[:, (KR + K0) // 16:(KR + K0 + K1) // 16],
                num_idxs=K1, num_idxs_reg=K1, elem_size=64,
                single_packet=False, queue_num=2)

            tt = nc.vector.tensor_tensor
            ts = nc.vector.tensor_scalar
            stt = nc.vector.scalar_tensor_tensor
            act = nc.scalar.activation

            def T(shape, tag):
                return pool.tile([128] + shape, f32, name=tag, tag=tag)

            # ---- reg extraction: 4-wide one-hot per (cell, win) ----
            ext = T([4, Jr, 5], "ext")
            grap = gt_reg[:].rearrange("p a b -> p (a b)")
            ohv = _V(oh4t[:], [[4, Jr], [0, 5], [1, 4]])
            for cw in range(4):
                cell, win = cw >> 1, cw & 1
                gv = _V(grap, [[128, Jr], [1, 5], [16, 4]],
                        extra_off=cell * 16 + win * 8)
                prod = T([Jr, 5, 4], f"prodr{cw}")
                tt(out=prod[:], in0=gv, in1=ohv, op=Alu.mult)
                nc.vector.tensor_reduce(out=ext[:, cw], in_=prod[:],
                                        axis=X, op=Alu.add)

            eap = ext[:].rearrange("p a b c -> p (a b c)")
            pobj_v = _V(eap, [[5 * Jr, 4], [5, Jr]], extra_off=0)
            pr01_v = _V(eap, [[1, 2], [5 * Jr, 4], [5, Jr]], extra_off=1)
            pr23_v = _V(eap, [[1, 2], [5 * Jr, 4], [5, Jr]], extra_off=3)

            def hfv(i, n=1):
                if n == 1:
                    return hf[:, i, :]
                return hf[:, i:i + n, :]

            def r4(apx):   # [128, 2, G2] -> [128, 2, 4, Jr]
                return apx.rearrange("p c (a b) -> p c a b", b=Jr)

            # ---- SIoU math ----
            t01 = T([2, G2], "t01")
            act(r4(t01[:]), pr01_v, Act.Tanh)
            sg = T([2, G2], "sg")
            act(r4(sg[:]), pr23_v, Act.Sigmoid)

            txy = T([2, G2], "txy")
            tt(out=txy[:], in0=t01[:], in1=hfv(F_GIJ, 2), op=Alu.add)
            b1a = T([2, G2], "b1a")
            stt(out=b1a[:], in0=sg[:], scalar=-80.0, in1=txy[:],
                op0=Alu.mult, op1=Alu.add)
            b1b = T([2, G2], "b1b")
            stt(out=b1b[:], in0=sg[:], scalar=80.0, in1=txy[:],
                op0=Alu.mult, op1=Alu.add)
            wh1 = T([2, G2], "wh1")
            tt(out=wh1[:], in0=b1b[:], in1=b1a[:], op=Alu.subtract)
            area1 = T([G2], "area1")
            tt(out=area1[:], in0=wh1[:, 0, :], in1=wh1[:, 1, :], op=Alu.mult)

            b2a = hfv(F_B2A, 2)
            b2b = hfv(F_B2B, 2)
            mn = T([2, G2], "mn")
            tt(out=mn[:], in0=b1b[:], in1=b2b, op=Alu.min)
            mx = T([2, G2], "mx")
            tt(out=mx[:], in0=b1a[:], in1=b2a, op=Alu.max)
            dcl = T([2, G2], "dcl")
            tt(out=dcl[:], in0=mn[:], in1=mx[:], op=Alu.subtract)
            ts(dcl[:], dcl[:], 0.0, None, Alu.max)
            inter = T([G2], "inter")
            tt(out=inter[:], in0=dcl[:, 0, :], in1=dcl[:, 1, :], op=Alu.mult)

            u = T([G2], "u")
            stt(out=u[:], in0=inter[:], scalar=-1.0, in1=area1[:],
                op0=Alu.mult, op1=Alu.add)
            tt(out=u[:], in0=u[:], in1=hfv(F_AREA2), op=Alu.add)
            invu = T([G2], "invu")
            nc.vector.reciprocal(invu[:], u[:])
            iou0 = T([G2], "iou0")
            tt(out=iou0[:], in0=inter[:], in1=invu[:], op=Alu.mult)

            mx2 = T([2, G2], "mx2")
            tt(out=mx2[:], in0=b1b[:], in1=b2b, op=Alu.max)
            mn2 = T([2, G2], "mn2")
            tt(out=mn2[:], in0=b1a[:], in1=b2a, op=Alu.min)
            cwh = T([2, G2], "cwh")
            tt(out=cwh[:], in0=mx2[:], in1=mn2[:], op=Alu.subtract)

            s2 = T([2, G2], "s2")
            tt(out=s2[:], in0=hfv(F_SXY, 2), in1=b1a[:], op=Alu.subtract)
            tt(out=s2[:], in0=s2[:], in1=b1b[:], op=Alu.subtract)
            sq = T([2, G2], "sq")
            act(sq[:], s2[:], Act.Square)
            ssum = T([G2], "ssum")
            tt(out=ssum[:], in0=sq[:, 0, :], in1=sq[:, 1, :], op=Alu.add)
            rs = T([G2], "rs")
            nc.vector.reciprocal(rs[:], ssum[:])
            invsig = T([G2], "invsig")
            act(invsig[:], rs[:], Act.Sqrt)

            sabs = T([2, G2], "sabs")
            act(sabs[:], s2[:], Act.Abs)
            sin12 = T([2, G2], "sin12")
            tt(out=sin12[:], in0=sabs[:],
               in1=_V(invsig[:], [[0, 2], [1, G2]]), op=Alu.mult)
            sina = T([G2], "sina")
            tt(out=sina[:], in0=sin12[:, 0, :], in1=sin12[:, 1, :], op=Alu.min)

            sa2 = T([G2], "sa2")
            tt(out=sa2[:], in0=sina[:], in1=sina[:], op=Alu.mult)
            om = T([G2], "om")
            ts(om[:], sa2[:], -1.0, 1.0, Alu.mult, Alu.add)
            rt = T([G2], "rt")
            act(rt[:], om[:], Act.Sqrt)
            gam4 = T([G2], "gam4")
            tt(out=gam4[:], in0=sina[:], in1=rt[:], op=Alu.mult)
            ts(gam4[:], gam4[:], 0.5, -0.5, Alu.mult, Alu.add)

            invcw = T([2, G2], "invcw")
            nc.vector.reciprocal(invcw[:], cwh[:])
            rr0 = T([2, G2], "rr0")
            tt(out=rr0[:], in0=s2[:], in1=invcw[:], op=Alu.mult)
            gr = T([2, G2], "gr")
            tt(out=gr[:], in0=rr0[:], in1=rr0[:], op=Alu.mult)
            tt(out=gr[:], in0=gr[:], in1=_V(gam4[:], [[0, 2], [1, G2]]),
               op=Alu.mult)
            eg = T([2, G2], "eg")
            act(eg[:], gr[:], Act.Exp)
            t_eg = T([G2], "t_eg")
            tt(out=t_eg[:], in0=eg[:, 0, :], in1=eg[:, 1, :], op=Alu.add)

            wh2t = hfv(F_WH2, 2)
            dwh = T([2, G2], "dwh")
            tt(out=dwh[:], in0=wh1[:], in1=wh2t, op=Alu.subtract)
            adwh = T([2, G2], "adwh")
            act(adwh[:], dwh[:], Act.Abs)
            mxw = T([2, G2], "mxw")
            tt(out=mxw[:], in0=wh1[:], in1=wh2t, op=Alu.max)
            nc.vector.reciprocal(mxw[:], mxw[:])
            omw = T([2, G2], "omw")
            tt(out=omw[:], in0=adwh[:], in1=mxw[:], op=Alu.mult)
            ewh = T([2, G2], "ewh")
            act(ewh[:], omw[:], Act.Exp, scale=-1.0)
            oe = T([2, G2], "oe")
            ts(oe[:], ewh[:], -1.0, 1.0, Alu.mult, Alu.add)
            tt(out=oe[:], in0=oe[:], in1=oe[:], op=Alu.mult)
            tt(out=oe[:], in0=oe[:], in1=oe[:], op=Alu.mult)
            shp = T([G2], "shp")
            tt(out=shp[:], in0=oe[:, 0, :], in1=oe[:, 1, :], op=Alu.add)

            c1 = T([G2], "c1")
            stt(out=c1[:], in0=shp[:], scalar=-1.0, in1=t_eg[:],
                op0=Alu.mult, op1=Alu.add)
            ts(c1[:], c1[:], 0.5, -1.0, Alu.mult, Alu.add)
            iou_v = out_t[:, 0:G2]
            tt(out=iou_v, in0=iou0[:], in1=c1[:], op=Alu.add)

            # sum(iou*m) partial per partition
            scr = T([G2], "scr")
            stt(out=scr[:], in0=iou_v, scalar=1.0, in1=hfv(F_M),
                op0=Alu.mult, op1=Alu.mult,
                accum_out=out_t[:, 3 * G2:3 * G2 + 1])

            # pobj for phase B
            nc.vector.tensor_copy(
                out=out_t[:, 2 * G2:3 * G2].rearrange(
                    "p (a b) -> p a b", b=Jr),
                in_=pobj_v)

            # ---- class extraction ----
            ohc = T([J2, 64], "ohc")
            tt(out=ohc[:], in0=_V(iot[:], [[0, J2], [1, 64]]),
               in1=_V(pht[:], [[1, J2], [0, 64]]), op=Alu.is_equal)
            ctv = _V(gt_cls[:], [[64, J2], [1, 64]])
            ctv1 = _V(gt_cls[:], [[64, J2], [1, 64]], extra_off=1)
            pg = T([2, J2], "pg")
            prodc = T([J2, 64], "prodc")
            tt(out=prodc[:], in0=ctv, in1=ohc[:], op=Alu.mult)
            nc.vector.tensor_reduce(out=pg[:, 0], in_=prodc[:],
                                    axis=X, op=Alu.add)
            prodd = T([J2, 64], "prodd")
            tt(out=prodd[:], in0=ctv1, in1=ohc[:], op=Alu.mult)
            nc.vector.tensor_reduce(out=pg[:, 1], in_=prodd[:],
                                    axis=X, op=Alu.add)
            ts(pg[:], pg[:], 1e-38, None, Alu.max)
            lnt = T([2, J2], "lnt")
            act(lnt[:], pg[:], Act.Ln)
            lnp_in = _V(lnt[:].rearrange("p a b -> p (a b)"),
                        [[J2, 2], [1, 2], [2, Jr]])
            tt(out=out_t[:, G2:2 * G2].rearrange(
                   "p (c w j) -> p c w j", c=2, w=2),
               in0=lnp_in,
               in1=hfv(F_MCLSV).rearrange("p (c w j) -> p c w j", c=2, w=2),
               op=Alu.mult)

            nc.sync.dma_start(out=aoutT.ap(), in_=out_t[:])

    lower_extended_insts(nc)
    _split_excess_waits(nc)
    return nc


# ---------------- phase B program ----------------

def _build_phase_b(meta):
    G2 = meta["G2"]
    AOUT = 3 * G2 + 8

    nc = bass.Bass("TRN2", debug=False)
    binT = nc.dram_tensor("bin", [128, AOUT], f32, kind="ExternalInput")
    hbT = nc.dram_tensor("hostb", [128, 3 * G2], f32, kind="ExternalInput")
    imT = nc.dram_tensor("imean", [128, 1], f32, kind="ExternalInput")
    poT = nc.dram_tensor("pobjd", [128, 400], f32, kind="ExternalInput")
    boutT = nc.dram_tensor("bout", [128, 8], f32, kind="ExternalOutput")

    with TileContext(nc) as tc:
        with tc.tile_pool(name="sbuf", bufs=1) as pool:
            nc.gpsimd.load_library(library_config.mlp)

            bi = pool.tile([128, AOUT], f32)
            nc.sync.dma_start(out=bi[:], in_=binT.ap())
            h2 = pool.tile([128, 3, G2], f32)
            nc.sync.dma_start(
                out=h2[:], in_=hbT.ap().rearrange("p (f g) -> p f g", f=3))
            im = pool.tile([128, 1], f32)
            nc.sync.dma_start(out=im[:], in_=imT.ap())
            pod = pool.tile([128, 400], f32)
            nc.sync.dma_start(out=pod[:], in_=poT.ap())

            ob = pool.tile([128, 8], f32)
            nc.vector.memset(ob[:], 0.0)

            iou_v = bi[:, 0:G2]
            lnp_v = bi[:, G2:2 * G2]
            pox = bi[:, 2 * G2:3 * G2]
            m_v, mkA, mkB = h2[:, 0, :], h2[:, 1, :], h2[:, 2, :]

            tt = nc.vector.tensor_tensor
            ts = nc.vector.tensor_scalar
            stt = nc.vector.scalar_tensor_tensor

            def T(shape, tag):
                return pool.tile([128] + shape, f32, name=tag, tag=tag)

            f_v = T([G2], "f")
            tt(out=f_v[:], in0=iou_v,
               in1=im[:].to_broadcast([128, G2]), op=Alu.is_gt)
            tt(out=f_v[:], in0=f_v[:], in1=m_v, op=Alu.mult)

            # bout0 = sum f*iou ; bout1 = sum f*lnp ; bout2 = sum f
            s0 = T([G2], "s0")
            stt(out=s0[:], in0=iou_v, scalar=1.0, in1=f_v[:],
                op0=Alu.mult, op1=Alu.mult, accum_out=ob[:, 0:1])
            s1 = T([G2], "s1")
            stt(out=s1[:], in0=lnp_v, scalar=1.0, in1=f_v[:],
                op0=Alu.mult, op1=Alu.mult, accum_out=ob[:, 1:2])
            nc.vector.tensor_reduce(out=ob[:, 2:3], in_=f_v[:],
                                    axis=X, op=Alu.add)

            # nperb (both batches of this core) -> all partitions
            np2 = pool.tile([128, 2], f32)
            sA = T([G2], "sA")
            stt(out=sA[:], in0=mkA, scalar=1.0, in1=f_v[:],
                op0=Alu.mult, op1=Alu.mult, accum_out=np2[:, 0:1])
            sB = T([G2], "sB")
            stt(out=sB[:], in0=mkB, scalar=1.0, in1=f_v[:],
                op0=Alu.mult, op1=Alu.mult, accum_out=np2[:, 1:2])
            npa = pool.tile([128, 2], f32)
            import concourse.bass_isa as bass_isa
            nc.gpsimd.partition_all_reduce(
                npa[:], np2[:], channels=128,
                reduce_op=bass_isa.ReduceOp.add)
            ts(npa[:], npa[:], 0.5, None, Alu.max)
            inv = pool.tile([128, 2], f32)
            nc.vector.reciprocal(inv[:], npa[:])
            ts(inv[:], inv[:], 6400.0, None, Alu.mult)

            fv = T([G2], "fv")
            ts(fv[:], mkA, inv[:, 0:1], None, Alu.mult)
            t3 = T([G2], "t3")
            ts(t3[:], mkB, inv[:, 1:2], None, Alu.mult)
            tt(out=fv[:], in0=fv[:], in1=t3[:], op=Alu.add)

            # obj correction: f*(sl1(pobj-iou)*fval - 0.375*pobj^2)
            d = T([G2], "d")
            tt(out=d[:], in0=pox, in1=iou_v, op=Alu.subtract)
            sqd = T([G2], "sqd")
            tt(out=sqd[:], in0=d[:], in1=d[:], op=Alu.mult)
            ad = T([G2], "ad")
            stt(out=ad[:], in0=d[:], scalar=-1.0, in1=d[:],
                op0=Alu.mult, op1=Alu.max)
            l_ = T([G2], "l_")
            ts(l_[:], ad[:], 0.5, None, Alu.subtract)
            cc = T([G2], "cc")
            ts(cc[:], ad[:], 1.0, None, Alu.is_lt)
            qd = T([G2], "qd")
            stt(out=qd[:], in0=sqd[:], scalar=0.5, in1=l_[:],
                op0=Alu.mult, op1=Alu.subtract)
            tt(out=qd[:], in0=cc[:], in1=qd[:], op=Alu.mult)
            tt(out=qd[:], in0=l_[:], in1=qd[:], op=Alu.add)
            tt(out=qd[:], in0=qd[:], in1=fv[:], op=Alu.mult)
            po2 = T([G2], "po2")
            stt(out=po2[:], in0=pox, scalar=-0.375, in1=pox,
                op0=Alu.mult, op1=Alu.mult)
            tt(out=qd[:], in0=qd[:], in1=po2[:], op=Alu.add)
            s4 = T([G2], "s4")
            stt(out=s4[:], in0=qd[:], scalar=1.0, in1=f_v[:],
                op0=Alu.mult, op1=Alu.mult, accum_out=ob[:, 3:4])

            # dense obj baseline partial: sum(pobj^2)
            s5 = pool.tile([128, 400], f32)
            stt(out=s5[:], in0=pod[:], scalar=1.0, in1=pod[:],
                op0=Alu.mult, op1=Alu.mult, accum_out=ob[:, 4:5])

            nc.sync.dma_start(out=boutT.ap(), in_=ob[:])

    lower_extended_insts(nc)
    _split_excess_waits(nc)
    return nc


# ---------------- host-side patches ----------------

def _sl1(x):
    ax = abs(x)
    return 0.5 * x * x if ax < 1.0 else ax - 0.5


def _host_patches(per_core, meta, aouts, iou_mean):
    """Returns (corr_patch, s2_patch): corr_patch is subtracted from the
    device obj-corr sum (collision losers); s2_patch is added to the
    device sum f*lnp (class phi=63 crossings)."""
    G2 = meta["G2"]
    corr_patch = 0.0
    s2_patch = 0.0
    for k, d in enumerate(per_core):
        a = aouts[k]
        iou = a[:, 0:G2]
        pobj = a[:, 2 * G2:3 * G2]
        hb = d["hostb"].reshape(128, 3, G2)
        m = hb[:, 0, :]
        mkB = hb[:, 2, :]
        f = (iou > iou_mean) & (m > 0)

        # nperb for this core's two batches (exact integer counts)
        npA = max(float((f & (mkB < 0.5)).sum()), 0.5)
        npB = max(float((f & (mkB > 0.5)).sum()), 0.5)
        fvalA = 6400.0 / npA
        fvalB = 6400.0 / npB

        # collision dedup: group f-positive candidates by cell id
        cells = d["candcell"]
        fpos = f & (cells >= 0)
        if fpos.any():
            cid = cells[fpos]
            orig = d["candorig"][fpos]
            iouv = iou[fpos]
            pov = pobj[fpos]
            isB = mkB[fpos] > 0.5
            order = np.argsort(cid, kind="stable")
            cid, orig, iouv, pov, isB = (cid[order], orig[order],
                                         iouv[order], pov[order], isB[order])
            i = 0
            n = len(cid)
            while i < n:
                jx = i
                while jx + 1 < n and cid[jx + 1] == cid[i]:
                    jx += 1
                if jx > i:
                    widx = i + int(np.argmax(orig[i:jx + 1]))
                    for t in range(i, jx + 1):
                        if t == widx:
                            continue
                        fval = fvalB if isB[t] else fvalA
                        corr_patch += (_sl1(float(pov[t]) - float(iouv[t]))
                                       * fval - 0.375 * float(pov[t]) ** 2)
                i = jx + 1

        # class crossing patch
        for (p, col, pv) in d["crossing"]:
            if f[p, col]:
                s2_patch += math.log(max(pv, 1e-38))
    return corr_patch, s2_patch


# ---------------- main entry ----------------

_CACHE = {}


def kernel(preds, targets):
    per_core, meta = _prep(preds, targets)

    key = (meta["J0"], meta["J1"])
    if key not in _CACHE:
        _CACHE[key] = (_build_phase_a(meta), _build_phase_b(meta))
    nc_a, nc_b = _CACHE[key]

    core_ids = list(range(NCORES))
    in_maps_a = [dict(regarr=d["regarr"], clsarr=d["clsarr"],
                      idx16=d["idx16"], hostf=d["hostf"], oh4=d["oh4"],
                      phic=d["phic"], iota=meta["iota"]) for d in per_core]
    res_a = run_bass_kernel_spmd(nc_a, in_maps_a, core_ids)

    G2 = meta["G2"]
    aouts = [res_a.results[k]["aout"] for k in core_ids]
    sum_im = sum(float(a[:, 3 * G2].sum(dtype=np.float64)) for a in aouts)
    iou_mean = np.float32(np.float32(sum_im) / np.float32(meta["cnt_m"]))

    imean_arr = np.full((128, 1), iou_mean, np.float32)
    in_maps_b = [dict(bin=aouts[k], hostb=per_core[k]["hostb"],
                      imean=imean_arr, pobjd=per_core[k]["pobjd"])
                 for k in core_ids]
    res_b = run_bass_kernel_spmd(nc_b, in_maps_b, core_ids)

    bouts = [res_b.results[k]["bout"] for k in core_ids]
    Sfi = sum(float(o[:, 0].sum(dtype=np.float64)) for o in bouts)
    S2 = sum(float(o[:, 1].sum(dtype=np.float64)) for o in bouts)
    cnt_f = max(sum(float(o[:, 2].sum(dtype=np.float64)) for o in bouts), 1.0)
    corr = sum(float(o[:, 3].sum(dtype=np.float64)) for o in bouts)
    base = sum(float(o[:, 4].sum(dtype=np.float64)) for o in bouts)

    corr_patch, s2_patch = _host_patches(per_core, meta, aouts, iou_mean)

    iou_loss = np.float32((cnt_f - Sfi) / cnt_f)
    cls_loss = np.float32(-(S2 + s2_patch) / cnt_f)
    obj_loss = np.float32((0.375 * base + corr - corr_patch) / (N * HW))
    loss = np.float32(iou_loss * 8 + obj_loss * 16 + cls_loss)
    return (iou_loss, obj_loss, cls_loss, loss)


# revision 10
# speedup vs baseline: 1.9913x; 1.0538x over previous
"""Trainium2 Bass kernel for nn_DetectorLoss (SIoU detector loss).

Strategy: data-parallel over batch N=16 -> 8 cores x 2 batches.

Host re-lays preds (input-independent permutations only):
  - regarr: per cell r a 16-float record [ch0..4 @ r | pad | ch0..4 @ r+160 | pad]
    so ONE 256B-aligned dma_gather descriptor pair covers all 4 quadrant
    candidates' obj+reg channels of a ground truth (window of 30 floats at
    16*r0, phase in {0,16,32,48} -> 4-wide one-hot extraction).
  - clsarr: plain [80, HW] class channels per batch; one 64-float row per
    (GT, y-row) covers both x cells; 64-wide one-hot extraction.

Phase A computes per-candidate SIoU iou, log-class prob, pobj and the
partial sum(iou*m); host combines the global iou_mean; phase B applies the
f-mask, computes the masked reductions and the dense obj baseline.
Cell-collision dedup (rare) and phi=63 class-row crossings (rare) are
patched exactly on host from the per-candidate outputs.
"""

import math
import numpy as np

import concourse.bass as bass
import concourse.mybir as mybir
from concourse import library_config
from concourse.bass import AP
from concourse.library_overlay import lower_extended_insts
from concourse.tile import TileContext
from concourse.bass_utils import run_bass_kernel_spmd

# ---------------- problem constants (hardcoded per spec) ----------------
N, C, H, W = 16, 85, 160, 160
HW = H * W                  # 25600
NCORES = 8
BPC = 2
M_DEFAULT = 4096

f32 = mybir.dt.float32
i16 = mybir.dt.int16
Alu = mybir.AluOpType
Act = mybir.ActivationFunctionType
X = mybir.AxisListType.X

REGROWS = BPC * HW * 16 // 64      # 12800
CLSROWS = 80 * HW // 64            # 32000 per batch

# hostf field indices
F_GIJ = 0      # 2
F_B2A = 2      # 2
F_B2B = 4      # 2
F_SXY = 6      # 2
F_WH2 = 8      # 2
F_AREA2 = 10
F_M = 11
F_MCLSV = 12
NF = 13

MAX_WAITS = 1


def _split_excess_waits(nc):
    """This neuronxcc build rejects >1 sem wait on several instruction
    classes; hoist extras onto same-engine Drain carriers placed before."""
    for f in nc.m.functions:
        for bb in f.blocks:
            new_list = []
            for ins in bb.instructions:
                si = ins.sync_info
                if si is not None and len(si.on_wait) > MAX_WAITS:
                    waits = list(si.on_wait)
                    excess, keep = waits[:-MAX_WAITS], waits[-MAX_WAITS:]
                    while excess:
                        chunk, excess = excess[:MAX_WAITS], excess[MAX_WAITS:]
                        carrier = mybir.InstDrain(
                            name=nc.get_next_instruction_name(),
                            engine=ins.engine, ins=[], outs=[],
                            bass_is_fusable=False,
                            sync_info=mybir.SyncInfo(on_wait=chunk, on_update=[]),
                        )
                        nc.register_instruction(carrier)
                        new_list.append(carrier)
                    si.on_wait = keep
                new_list.append(ins)
            bb.instructions[:] = new_list


def _V(tap, dims, extra_off=0):
    """Custom free-dim view of a tile AP (keeps the partition dim)."""
    return AP(tensor=tap.tensor, offset=tap.offset + extra_off,
              ap=[list(tap.ap[0])] + [list(d) for d in dims])


def _wrap16(idxs):
    n = idxs.shape[0]
    base16 = idxs.reshape(n // 16, 16).T.astype(np.int16)
    return np.tile(base16, (8, 1))


# ---------------- host preparation ----------------

def _prep(preds, targets):
    preds = np.asarray(preds, np.float32)
    targets = np.asarray(targets, np.float32)
    M = targets.shape[0]
    dt = np.float32

    scale = np.array([1, 1, W, H, W, H], dt)
    gt = (targets * scale).astype(dt)
    x0 = gt[:, 2].astype(np.int32)
    y0 = gt[:, 3].astype(np.int32)
    quad = np.array([[0, 0], [1, 0], [0, 1], [1, 1]], np.int32)
    gijx = x0[None, :] + quad[:, 0:1]
    gijy = y0[None, :] + quad[:, 1:2]
    m4 = (np.minimum(np.where(gijx < H, gijx, 0),
                     np.where(gijy < H, gijy, 0)) > 0)      # [4, M]
    b = targets[:, 0].astype(np.int32)
    gcls = targets[:, 1].astype(np.int32)

    gx, gy, gw, gh = gt[:, 2], gt[:, 3], gt[:, 4], gt[:, 5]
    half = dt(0.5)
    b2x1 = (gx - gw * half).astype(dt)
    b2x2 = (gx + gw * half).astype(dt)
    b2y1 = (gy - gh * half).astype(dt)
    b2y2 = (gy + gh * half).astype(dt)
    w2 = (b2x2 - b2x1).astype(dt)
    h2 = ((b2y2 - b2y1) + dt(1e-7)).astype(dt)
    area2h = (w2 * h2).astype(dt)
    sx2 = (b2x1 + b2x2).astype(dt)
    sy2 = (b2y1 + b2y2).astype(dt)

    cnt_m = max(int(m4.sum()), 1)
    r0 = (y0.astype(np.int64) * W + x0)
    core = b >> 1
    lbv_all = b & 1

    cnts = np.zeros((NCORES, 2), np.int64)
    for k in range(NCORES):
        cnts[k, 0] = int(((core == k) & (lbv_all == 0)).sum())
        cnts[k, 1] = int(((core == k) & (lbv_all == 1)).sum())
    J0 = int(max(1, math.ceil(cnts[:, 0].max() / 128)))
    J1 = int(max(1, math.ceil(cnts[:, 1].max() / 128)))
    Jr = J0 + J1
    G2 = 4 * Jr
    J2 = 2 * Jr

    per_core = []
    for k in range(NCORES):
        pc = preds[BPC * k:BPC * (k + 1)]
        reg = np.zeros((BPC, HW, 16), dt)
        t5 = pc[:, 0:5].reshape(BPC, 5, HW).transpose(0, 2, 1)
        reg[:, :, 0:5] = t5
        reg[:, :-W, 8:13] = t5[:, W:, :]
        clsarr = np.ascontiguousarray(pc[:, 5:85]).reshape(-1)
        pobjd = np.ascontiguousarray(pc[:, 0]).reshape(128, 400)

        hostf = np.zeros((128, NF, G2), dt)
        hostf[:, F_B2B:F_B2B + 2] = 1.0
        hostf[:, F_SXY:F_SXY + 2] = 1.0
        hostf[:, F_WH2:F_WH2 + 2] = 1.0
        hostf[:, F_AREA2] = 1.0
        oh4 = np.zeros((128, Jr, 4), dt)
        phic = np.full((128, J2), -1.0, dt)
        regg = np.zeros((2 * Jr, 128), np.int64)
        clsg0 = np.zeros((2 * J0, 128), np.int64)
        clsg1 = np.zeros((2 * J1, 128), np.int64)
        candcell = np.full((128, G2), -1, np.int64)
        candorig = np.full((128, G2), -1, np.int64)
        hostb = np.zeros((128, 3, G2), dt)
        crossing = []

        for lbv in (0, 1):
            gl = np.where((core == k) & (lbv_all == lbv))[0]
            joff = 0 if lbv == 0 else J0
            cg = clsg0 if lbv == 0 else clsg1
            for i, g in enumerate(gl):
                p = i % 128
                jrel = i // 128
                j = jrel + joff
                rr = int(r0[g])
                s = rr & 3
                bb0 = lbv * 6400 + (rr >> 2)
                regg[2 * j, p] = bb0
                regg[2 * j + 1, p] = min(bb0 + 1, REGROWS - 1)
                oh4[p, j, s] = 1.0
                for win in (0, 1):
                    yy = int(y0[g]) + win
                    if yy <= H - 1:
                        flat = int(gcls[g]) * HW + yy * W + int(x0[g])
                        cg[jrel * 2 + win, p] = flat >> 6
                        phic[p, j * 2 + win] = dt(flat & 63)
                for cell in (0, 1):
                    for win in (0, 1):
                        cw = cell * 2 + win
                        col = cw * Jr + j
                        q = win * 2 + cell
                        mm = bool(m4[q, g])
                        gi = (int(x0[g]) + cell) if mm else 0
                        gj = (int(y0[g]) + win) if mm else 0
                        hostf[p, F_GIJ + 0, col] = gi
                        hostf[p, F_GIJ + 1, col] = gj
                        hostf[p, F_M, col] = 1.0 if mm else 0.0
                        hostf[p, F_B2A + 0, col] = b2x1[g]
                        hostf[p, F_B2A + 1, col] = b2y1[g]
                        hostf[p, F_B2B + 0, col] = b2x2[g]
                        hostf[p, F_B2B + 1, col] = b2y2[g]
                        hostf[p, F_SXY + 0, col] = sx2[g]
                        hostf[p, F_SXY + 1, col] = sy2[g]
                        hostf[p, F_WH2 + 0, col] = w2[g]
                        hostf[p, F_WH2 + 1, col] = h2[g]
                        hostf[p, F_AREA2, col] = area2h[g]
                        hostf[p, F_MCLSV, col] = 1.0 if mm else 0.0
                        hostb[p, 0, col] = 1.0 if mm else 0.0
                        hostb[p, 1, col] = 1.0 - lbv
                        hostb[p, 2, col] = float(lbv)
                        candorig[p, col] = q * M + g
                        if mm:
                            candcell[p, col] = (int(b[g]) * HW + gj * W + gi)
                            if cell == 1:
                                yy = int(y0[g]) + win
                                flat = (int(gcls[g]) * HW + yy * W
                                        + int(x0[g]))
                                if (flat & 63) == 63:
                                    hostf[p, F_MCLSV, col] = 0.0
                                    pv = float(preds[BPC * k + lbv,
                                               5 + int(gcls[g]), yy,
                                               int(x0[g]) + 1])
                                    crossing.append((p, col, pv))

        idx16 = np.concatenate([
            _wrap16(clsg0.reshape(-1)),
            _wrap16(clsg1.reshape(-1)),
            _wrap16(regg.reshape(-1)),
        ], axis=1)

        # host-side class one-hot [128, J2, 64]
        ohc = (np.arange(64, dtype=dt)[None, None, :]
               == phic[:, :, None]).astype(dt)

        big = np.concatenate([
            hostf.reshape(128, NF * G2),
            oh4.reshape(128, Jr * 4),
            ohc.reshape(128, J2 * 64),
        ], axis=1)

        per_core.append(dict(
            regarr=reg.reshape(-1), clsarr=clsarr, pobjd=pobjd,
            idx16=idx16, big=big,
            hostb=hostb.reshape(128, 3, G2),
            candcell=candcell, candorig=candorig, crossing=crossing,
        ))

    meta = dict(J0=J0, J1=J1, Jr=Jr, G2=G2, J2=J2, cnt_m=cnt_m, M=M)
    return per_core, meta


# ---------------- phase A program ----------------

def _build_phase_a(meta):
    J0, J1 = meta["J0"], meta["J1"]
    Jr, G2, J2 = meta["Jr"], meta["G2"], meta["J2"]
    KR = 2 * Jr * 128
    K0 = 2 * J0 * 128
    K1 = 2 * J1 * 128
    KTW = (KR + K0 + K1) // 16
    BIGW = NF * G2 + Jr * 4 + J2 * 64
    AOUT = 3 * G2 + 8

    nc = bass.Bass("TRN2", debug=False, num_swdge_queues=4)
    regT = nc.dram_tensor("regarr", [BPC * HW * 16], f32, kind="ExternalInput")
    clsT = nc.dram_tensor("clsarr", [BPC * 80 * HW], f32, kind="ExternalInput")
    idxT = nc.dram_tensor("idx16", [128, KTW], i16, kind="ExternalInput")
    bigT = nc.dram_tensor("big", [128, BIGW], f32, kind="ExternalInput")
    aoutT = nc.dram_tensor("aout", [128, AOUT], f32, kind="ExternalOutput")

    with TileContext(nc) as tc:
        with tc.tile_pool(name="sbuf", bufs=1) as pool:
            nc.gpsimd.load_library(library_config.mlp)

            idx_t = pool.tile([128, KTW], i16)
            nc.sync.dma_start(out=idx_t[:], in_=idxT.ap())
            big = pool.tile([128, BIGW], f32)
            nc.sync.dma_start(out=big[:], in_=bigT.ap())
            hf = big[:, 0:NF * G2].rearrange("p (f g) -> p f g", f=NF)
            oh4v = big[:, NF * G2:NF * G2 + Jr * 4]
            ohcv = big[:, NF * G2 + Jr * 4:BIGW].rearrange(
                "p (a b) -> p a b", b=64)

            out_t = pool.tile([128, AOUT], f32)
            nc.vector.memset(out_t[:], 0.0)

            # ---- gathers: class first (feeds the first DVE block) ----
            gt_cls = pool.tile([128, J2 * 64 + 4], f32)
            nc.gpsimd.dma_gather(
                out_ap=gt_cls[:, 0:2 * J0 * 64].rearrange(
                    "p (a b) -> p a b", b=64),
                in_ap=clsT.ap()[0:80 * HW].rearrange("(r e) -> r e", e=64),
                idxs_ap=idx_t[:, 0:K0 // 16],
                num_idxs=K0, num_idxs_reg=K0, elem_size=64,
                single_packet=False, queue_num=0)
            nc.gpsimd.dma_gather(
                out_ap=gt_cls[:, 2 * J0 * 64:J2 * 64].rearrange(
                    "p (a b) -> p a b", b=64),
                in_ap=clsT.ap()[80 * HW:].rearrange("(r e) -> r e", e=64),
                idxs_ap=idx_t[:, K0 // 16:(K0 + K1) // 16],
                num_idxs=K1, num_idxs_reg=K1, elem_size=64,
                single_packet=False, queue_num=1)
            gt_reg = pool.tile([128, 2 * Jr, 64], f32)
            nc.gpsimd.dma_gather(
                out_ap=gt_reg[:],
                in_ap=regT.ap().rearrange("(r e) -> r e", e=64),
                idxs_ap=idx_t[:, (K0 + K1) // 16:(K0 + K1 + KR) // 16],
                num_idxs=KR, num_idxs_reg=KR, elem_size=64,
                single_packet=False, queue_num=2)

            tt = nc.vector.tensor_tensor
            ts = nc.vector.tensor_scalar
            stt = nc.vector.scalar_tensor_tensor
            act = nc.scalar.activation

            def T(shape, tag):
                return pool.tile([128] + shape, f32, name=tag, tag=tag)

            def hfv(i, n=1):
                if n == 1:
                    return hf[:, i, :]
                return hf[:, i:i + n, :]

            # ---- class extraction (first DVE block) ----
            ctv = _V(gt_cls[:], [[64, J2], [1, 64]])
            ctv1 = _V(gt_cls[:], [[64, J2], [1, 64]], extra_off=1)
            pg = T([2, J2], "pg")
            prodc = T([J2, 64], "prodc")
            tt(out=prodc[:], in0=ctv, in1=ohcv, op=Alu.mult)
            nc.vector.tensor_reduce(out=pg[:, 0], in_=prodc[:],
                                    axis=X, op=Alu.add)
            prodd = T([J2, 64], "prodd")
            tt(out=prodd[:], in0=ctv1, in1=ohcv, op=Alu.mult)
            nc.vector.tensor_reduce(out=pg[:, 1], in_=prodd[:],
                                    axis=X, op=Alu.add)
            ts(pg[:], pg[:], 1e-38, None, Alu.max)
            lnt = T([2, J2], "lnt")
            act(lnt[:], pg[:], Act.Ln)
            lnp_in = _V(lnt[:].rearrange("p a b -> p (a b)"),
                        [[J2, 2], [1, 2], [2, Jr]])
            tt(out=out_t[:, G2:2 * G2].rearrange(
                   "p (c w j) -> p c w j", c=2, w=2),
               in0=lnp_in,
               in1=hfv(F_MCLSV).rearrange("p (c w j) -> p c w j", c=2, w=2),
               op=Alu.mult)

            # ---- reg extraction: 4-wide one-hot per (cell, win) ----
            ext = T([4, Jr, 5], "ext")
            grap = gt_reg[:].rearrange("p a b -> p (a b)")
            ohv = _V(oh4v, [[4, Jr], [0, 5], [1, 4]])
            for cw in range(4):
                cell, win = cw >> 1, cw & 1
                gv = _V(grap, [[128, Jr], [1, 5], [16, 4]],
                        extra_off=cell * 16 + win * 8)
                prod = T([Jr, 5, 4], f"prodr{cw}")
                tt(out=prod[:], in0=gv, in1=ohv, op=Alu.mult)
                nc.vector.tensor_reduce(out=ext[:, cw], in_=prod[:],
                                        axis=X, op=Alu.add)

            eap = ext[:].rearrange("p a b c -> p (a b c)")
            pobj_v = _V(eap, [[5 * Jr, 4], [5, Jr]], extra_off=0)
            pr01_v = _V(eap, [[1, 2], [5 * Jr, 4], [5, Jr]], extra_off=1)
            pr23_v = _V(eap, [[1, 2], [5 * Jr, 4], [5, Jr]], extra_off=3)

            def hfv(i, n=1):
                if n == 1:
                    return hf[:, i, :]
                return hf[:, i:i + n, :]

            def r4(apx):   # [128, 2, G2] -> [128, 2, 4, Jr]
                return apx.rearrange("p c (a b) -> p c a b", b=Jr)

            # ---- SIoU math ----
            t01 = T([2, G2], "t01")
            act(r4(t01[:]), pr01_v, Act.Tanh)
            sg = T([2, G2], "sg")
            act(r4(sg[:]), pr23_v, Act.Sigmoid)

            txy = T([2, G2], "txy")
            tt(out=txy[:], in0=t01[:], in1=hfv(F_GIJ, 2), op=Alu.add)
            b1a = T([2, G2], "b1a")
            stt(out=b1a[:], in0=sg[:], scalar=-80.0, in1=txy[:],
                op0=Alu.mult, op1=Alu.add)
            b1b = T([2, G2], "b1b")
            stt(out=b1b[:], in0=sg[:], scalar=80.0, in1=txy[:],
                op0=Alu.mult, op1=Alu.add)
            wh1 = T([2, G2], "wh1")
            tt(out=wh1[:], in0=b1b[:], in1=b1a[:], op=Alu.subtract)
            area1 = T([G2], "area1")
            tt(out=area1[:], in0=wh1[:, 0, :], in1=wh1[:, 1, :], op=Alu.mult)

            b2a = hfv(F_B2A, 2)
            b2b = hfv(F_B2B, 2)
            mn = T([2, G2], "mn")
            tt(out=mn[:], in0=b1b[:], in1=b2b, op=Alu.min)
            mx = T([2, G2], "mx")
            tt(out=mx[:], in0=b1a[:], in1=b2a, op=Alu.max)
            dcl = T([2, G2], "dcl")
            tt(out=dcl[:], in0=mn[:], in1=mx[:], op=Alu.subtract)
            ts(dcl[:], dcl[:], 0.0, None, Alu.max)
            inter = T([G2], "inter")
            tt(out=inter[:], in0=dcl[:, 0, :], in1=dcl[:, 1, :], op=Alu.mult)

            u = T([G2], "u")
            stt(out=u[:], in0=inter[:], scalar=-1.0, in1=area1[:],
                op0=Alu.mult, op1=Alu.add)
            tt(out=u[:], in0=u[:], in1=hfv(F_AREA2), op=Alu.add)
            invu = T([G2], "invu")
            nc.vector.reciprocal(invu[:], u[:])
            iou0 = T([G2], "iou0")
            tt(out=iou0[:], in0=inter[:], in1=invu[:], op=Alu.mult)

            mx2 = T([2, G2], "mx2")
            tt(out=mx2[:], in0=b1b[:], in1=b2b, op=Alu.max)
            mn2 = T([2, G2], "mn2")
            tt(out=mn2[:], in0=b1a[:], in1=b2a, op=Alu.min)
            cwh = T([2, G2], "cwh")
            tt(out=cwh[:], in0=mx2[:], in1=mn2[:], op=Alu.subtract)

            s2 = T([2, G2], "s2")
            tt(out=s2[:], in0=hfv(F_SXY, 2), in1=b1a[:], op=Alu.subtract)
            tt(out=s2[:], in0=s2[:], in1=b1b[:], op=Alu.subtract)
            sq = T([2, G2], "sq")
            act(sq[:], s2[:], Act.Square)
            ssum = T([G2], "ssum")
            tt(out=ssum[:], in0=sq[:, 0, :], in1=sq[:, 1, :], op=Alu.add)
            rs = T([G2], "rs")
            nc.vector.reciprocal(rs[:], ssum[:])
            invsig = T([G2], "invsig")
            act(invsig[:], rs[:], Act.Sqrt)

            sabs = T([2, G2], "sabs")
            act(sabs[:], s2[:], Act.Abs)
            sin12 = T([2, G2], "sin12")
            tt(out=sin12[:], in0=sabs[:],
               in1=_V(invsig[:], [[0, 2], [1, G2]]), op=Alu.mult)
            sina = T([G2], "sina")
            tt(out=sina[:], in0=sin12[:, 0, :], in1=sin12[:, 1, :], op=Alu.min)

            sa2 = T([G2], "sa2")
            tt(out=sa2[:], in0=sina[:], in1=sina[:], op=Alu.mult)
            om = T([G2], "om")
            ts(om[:], sa2[:], -1.0, 1.0, Alu.mult, Alu.add)
            rt = T([G2], "rt")
            act(rt[:], om[:], Act.Sqrt)
            gam4 = T([G2], "gam4")
            tt(out=gam4[:], in0=sina[:], in1=rt[:], op=Alu.mult)
            ts(gam4[:], gam4[:], 0.5, -0.5, Alu.mult, Alu.add)

            invcw = T([2, G2], "invcw")
            nc.vector.reciprocal(invcw[:], cwh[:])
            rr0 = T([2, G2], "rr0")
            tt(out=rr0[:], in0=s2[:], in1=invcw[:], op=Alu.mult)
            gr = T([2, G2], "gr")
            tt(out=gr[:], in0=rr0[:], in1=rr0[:], op=Alu.mult)
            tt(out=gr[:], in0=gr[:], in1=_V(gam4[:], [[0, 2], [1, G2]]),
               op=Alu.mult)
            eg = T([2, G2], "eg")
            act(eg[:], gr[:], Act.Exp)
            t_eg = T([G2], "t_eg")
            tt(out=t_eg[:], in0=eg[:, 0, :], in1=eg[:, 1, :], op=Alu.add)

            wh2t = hfv(F_WH2, 2)
            dwh = T([2, G2], "dwh")
            tt(out=dwh[:], in0=wh1[:], in1=wh2t, op=Alu.subtract)
            adwh = T([2, G2], "adwh")
            act(adwh[:], dwh[:], Act.Abs)
            mxw = T([2, G2], "mxw")
            tt(out=mxw[:], in0=wh1[:], in1=wh2t, op=Alu.max)
            nc.vector.reciprocal(mxw[:], mxw[:])
            omw = T([2, G2], "omw")
            tt(out=omw[:], in0=adwh[:], in1=mxw[:], op=Alu.mult)
            ewh = T([2, G2], "ewh")
            act(ewh[:], omw[:], Act.Exp, scale=-1.0)
            oe = T([2, G2], "oe")
            ts(oe[:], ewh[:], -1.0, 1.0, Alu.mult, Alu.add)
            tt(out=oe[:], in0=oe[:], in1=oe[:], op=Alu.mult)
            tt(out=oe[:], in0=oe[:], in1=oe[:], op=Alu.mult)
            shp = T([G2], "shp")
            tt(out=shp[:], in0=oe[:, 0, :], in1=oe[:, 1, :], op=Alu.add)

            c1 = T([G2], "c1")
            stt(out=c1[:], in0=shp[:], scalar=-1.0, in1=t_eg[:],
                op0=Alu.mult, op1=Alu.add)
            ts(c1[:], c1[:], 0.5, -1.0, Alu.mult, Alu.add)
            iou_v = out_t[:, 0:G2]
            tt(out=iou_v, in0=iou0[:], in1=c1[:], op=Alu.add)

            # sum(iou*m) partial per partition
            scr = T([G2], "scr")
            stt(out=scr[:], in0=iou_v, scalar=1.0, in1=hfv(F_M),
                op0=Alu.mult, op1=Alu.mult,
                accum_out=out_t[:, 3 * G2:3 * G2 + 1])

            # pobj for phase B
            nc.vector.tensor_copy(
                out=out_t[:, 2 * G2:3 * G2].rearrange(
                    "p (a b) -> p a b", b=Jr),
                in_=pobj_v)

            nc.sync.dma_start(out=aoutT.ap(), in_=out_t[:])

    lower_extended_insts(nc)
    _split_excess_waits(nc)
    return nc


# ---------------- phase B program ----------------

def _build_phase_b(meta):
    G2 = meta["G2"]
    AOUT = 3 * G2 + 8
    # merged input: [aout | m | fv | imean | pobjd]
    BINW = AOUT + G2 + G2 + 1 + 400

    nc = bass.Bass("TRN2", debug=False)
    binT = nc.dram_tensor("binall", [128, BINW], f32, kind="ExternalInput")
    boutT = nc.dram_tensor("bout", [128, 8], f32, kind="ExternalOutput")

    with TileContext(nc) as tc:
        with tc.tile_pool(name="sbuf", bufs=1) as pool:
            bi = pool.tile([128, BINW], f32)
            nc.sync.dma_start(out=bi[:], in_=binT.ap())

            ob = pool.tile([128, 8], f32)
            nc.vector.memset(ob[:], 0.0)

            iou_v = bi[:, 0:G2]
            lnp_v = bi[:, G2:2 * G2]
            pox = bi[:, 2 * G2:3 * G2]
            m_v = bi[:, AOUT:AOUT + G2]
            fv = bi[:, AOUT + G2:AOUT + 2 * G2]
            im = bi[:, AOUT + 2 * G2:AOUT + 2 * G2 + 1]
            pod = bi[:, AOUT + 2 * G2 + 1:BINW]

            tt = nc.vector.tensor_tensor
            ts = nc.vector.tensor_scalar
            stt = nc.vector.scalar_tensor_tensor

            def T(shape, tag):
                return pool.tile([128] + shape, f32, name=tag, tag=tag)

            f_v = T([G2], "f")
            tt(out=f_v[:], in0=iou_v,
               in1=im.to_broadcast([128, G2]), op=Alu.is_gt)
            tt(out=f_v[:], in0=f_v[:], in1=m_v, op=Alu.mult)

            # bout0 = sum f*iou ; bout1 = sum f*lnp ; bout2 = sum f
            s0 = T([G2], "s0")
            stt(out=s0[:], in0=iou_v, scalar=1.0, in1=f_v[:],
                op0=Alu.mult, op1=Alu.mult, accum_out=ob[:, 0:1])
            s1 = T([G2], "s1")
            stt(out=s1[:], in0=lnp_v, scalar=1.0, in1=f_v[:],
                op0=Alu.mult, op1=Alu.mult, accum_out=ob[:, 1:2])
            nc.vector.tensor_reduce(out=ob[:, 2:3], in_=f_v[:],
                                    axis=X, op=Alu.add)

            # obj correction: f*(sl1(pobj-iou)*fval - 0.375*pobj^2)
            d = T([G2], "d")
            tt(out=d[:], in0=pox, in1=iou_v, op=Alu.subtract)
            sqd = T([G2], "sqd")
            tt(out=sqd[:], in0=d[:], in1=d[:], op=Alu.mult)
            ad = T([G2], "ad")
            stt(out=ad[:], in0=d[:], scalar=-1.0, in1=d[:],
                op0=Alu.mult, op1=Alu.max)
            l_ = T([G2], "l_")
            ts(l_[:], ad[:], 0.5, None, Alu.subtract)
            cc = T([G2], "cc")
            ts(cc[:], ad[:], 1.0, None, Alu.is_lt)
            qd = T([G2], "qd")
            stt(out=qd[:], in0=sqd[:], scalar=0.5, in1=l_[:],
                op0=Alu.mult, op1=Alu.subtract)
            tt(out=qd[:], in0=cc[:], in1=qd[:], op=Alu.mult)
            tt(out=qd[:], in0=l_[:], in1=qd[:], op=Alu.add)
            tt(out=qd[:], in0=qd[:], in1=fv, op=Alu.mult)
            po2 = T([G2], "po2")
            stt(out=po2[:], in0=pox, scalar=-0.375, in1=pox,
                op0=Alu.mult, op1=Alu.mult)
            tt(out=qd[:], in0=qd[:], in1=po2[:], op=Alu.add)
            s4 = T([G2], "s4")
            stt(out=s4[:], in0=qd[:], scalar=1.0, in1=f_v[:],
                op0=Alu.mult, op1=Alu.mult, accum_out=ob[:, 3:4])

            # dense obj baseline partial: sum(pobj^2)
            s5 = pool.tile([128, 400], f32)
            stt(out=s5[:], in0=pod, scalar=1.0, in1=pod,
                op0=Alu.mult, op1=Alu.mult, accum_out=ob[:, 4:5])

            nc.sync.dma_start(out=boutT.ap(), in_=ob[:])

    lower_extended_insts(nc)
    _split_excess_waits(nc)
    return nc


# ---------------- host-side patches ----------------

def _sl1(x):
    ax = abs(x)
    return 0.5 * x * x if ax < 1.0 else ax - 0.5


def _host_patches(per_core, meta, aouts, iou_mean):
    """Returns (corr_patch, s2_patch): corr_patch is subtracted from the
    device obj-corr sum (collision losers); s2_patch is added to the
    device sum f*lnp (class phi=63 crossings)."""
    G2 = meta["G2"]
    corr_patch = 0.0
    s2_patch = 0.0
    for k, d in enumerate(per_core):
        a = aouts[k]
        iou = a[:, 0:G2]
        pobj = a[:, 2 * G2:3 * G2]
        hb = d["hostb"].reshape(128, 3, G2)
        m = hb[:, 0, :]
        mkB = hb[:, 2, :]
        f = (iou > iou_mean) & (m > 0)

        # nperb for this core's two batches (exact integer counts)
        npA = max(float((f & (mkB < 0.5)).sum()), 0.5)
        npB = max(float((f & (mkB > 0.5)).sum()), 0.5)
        fvalA = 6400.0 / npA
        fvalB = 6400.0 / npB

        # collision dedup: group f-positive candidates by cell id
        cells = d["candcell"]
        fpos = f & (cells >= 0)
        if fpos.any():
            cid = cells[fpos]
            orig = d["candorig"][fpos]
            iouv = iou[fpos]
            pov = pobj[fpos]
            isB = mkB[fpos] > 0.5
            order = np.argsort(cid, kind="stable")
            cid, orig, iouv, pov, isB = (cid[order], orig[order],
                                         iouv[order], pov[order], isB[order])
            i = 0
            n = len(cid)
            while i < n:
                jx = i
                while jx + 1 < n and cid[jx + 1] == cid[i]:
                    jx += 1
                if jx > i:
                    widx = i + int(np.argmax(orig[i:jx + 1]))
                    for t in range(i, jx + 1):
                        if t == widx:
                            continue
                        fval = fvalB if isB[t] else fvalA
                        corr_patch += (_sl1(float(pov[t]) - float(iouv[t]))
                                       * fval - 0.375 * float(pov[t]) ** 2)
                i = jx + 1

        # class crossing patch
        for (p, col, pv) in d["crossing"]:
            if f[p, col]:
                s2_patch += math.log(max(pv, 1e-38))
    return corr_patch, s2_patch


# ---------------- main entry ----------------

_CACHE = {}


def kernel(preds, targets):
    per_core, meta = _prep(preds, targets)

    key = (meta["J0"], meta["J1"])
    if key not in _CACHE:
        _CACHE[key] = (_build_phase_a(meta), _build_phase_b(meta))
    nc_a, nc_b = _CACHE[key]

    core_ids = list(range(NCORES))
    in_maps_a = [dict(regarr=d["regarr"], clsarr=d["clsarr"],
                      idx16=d["idx16"], big=d["big"]) for d in per_core]
    res_a = run_bass_kernel_spmd(nc_a, in_maps_a, core_ids)

    G2 = meta["G2"]
    aouts = [res_a.results[k]["aout"] for k in core_ids]
    sum_im = sum(float(a[:, 3 * G2].sum(dtype=np.float64)) for a in aouts)
    iou_mean = np.float32(np.float32(sum_im) / np.float32(meta["cnt_m"]))

    imean_arr = np.full((128, 1), iou_mean, np.float32)
    in_maps_b = []
    for k in core_ids:
        d = per_core[k]
        a = aouts[k]
        hb = d["hostb"]
        m_h, mkA, mkB = hb[:, 0], hb[:, 1], hb[:, 2]
        fh = (a[:, 0:G2] > iou_mean) & (m_h > 0)
        npA = max(float((fh & (mkA > 0.5)).sum()), 0.5)
        npB = max(float((fh & (mkB > 0.5)).sum()), 0.5)
        fv = (mkA * np.float32(6400.0 / npA)
              + mkB * np.float32(6400.0 / npB)).astype(np.float32)
        d["fvals"] = (6400.0 / npA, 6400.0 / npB)
        binall = np.concatenate(
            [a, m_h, fv, imean_arr, d["pobjd"]], axis=1).astype(np.float32)
        in_maps_b.append(dict(binall=binall))
    res_b = run_bass_kernel_spmd(nc_b, in_maps_b, core_ids)

    bouts = [res_b.results[k]["bout"] for k in core_ids]
    Sfi = sum(float(o[:, 0].sum(dtype=np.float64)) for o in bouts)
    S2 = sum(float(o[:, 1].sum(dtype=np.float64)) for o in bouts)
    cnt_f = max(sum(float(o[:, 2].sum(dtype=np.float64)) for o in bouts), 1.0)
    corr = sum(float(o[:, 3].sum(dtype=np.float64)) for o in bouts)
    base = sum(float(o[:, 4].sum(dtype=np.float64)) for o in bouts)

    corr_patch, s2_patch = _host_patches(per_core, meta, aouts, iou_mean)

    iou_loss = np.float32((cnt_f - Sfi) / cnt_f)
    cls_loss = np.float32(-(S2 + s2_patch) / cnt_f)
    obj_loss = np.float32((0.375 * base + corr - corr_patch) / (N * HW))
    loss = np.float32(iou_loss * 8 + obj_loss * 16 + cls_loss)
    return (iou_loss, obj_loss, cls_loss, loss)


# revision 13
# speedup vs baseline: 2.0169x; 1.0129x over previous
"""Trainium2 Bass kernel for nn_DetectorLoss (SIoU detector loss).

Strategy: data-parallel over batch N=16 -> 8 cores x 2 batches.

Host re-lays preds (input-independent permutations only):
  - regarr: per cell r a 16-float record [ch0..4 @ r | pad | ch0..4 @ r+160 | pad]
    so ONE 256B-aligned dma_gather descriptor pair covers all 4 quadrant
    candidates' obj+reg channels of a ground truth (window of 30 floats at
    16*r0, phase in {0,16,32,48} -> 4-wide one-hot extraction).
  - clsarr: plain [80, HW] class channels per batch; one 64-float row per
    (GT, y-row) covers both x cells; 64-wide one-hot extraction.

Phase A computes per-candidate SIoU iou, log-class prob, pobj and the
partial sum(iou*m); host combines the global iou_mean; phase B applies the
f-mask, computes the masked reductions and the dense obj baseline.
Cell-collision dedup (rare) and phi=63 class-row crossings (rare) are
patched exactly on host from the per-candidate outputs.
"""

import math
import numpy as np

import concourse.bass as bass
import concourse.mybir as mybir
from concourse import library_config
from concourse.bass import AP
from concourse.library_overlay import lower_extended_insts
from concourse.tile import TileContext
from concourse.bass_utils import run_bass_kernel_spmd

# ---------------- problem constants (hardcoded per spec) ----------------
N, C, H, W = 16, 85, 160, 160
HW = H * W                  # 25600
NCORES = 8
BPC = 2
M_DEFAULT = 4096

f32 = mybir.dt.float32
i16 = mybir.dt.int16
Alu = mybir.AluOpType
Act = mybir.ActivationFunctionType
X = mybir.AxisListType.X

REGROWS = BPC * HW * 16 // 64      # 12800
CLSROWS = 80 * HW // 64            # 32000 per batch

# hostf field indices
F_GIJ = 0      # 2
F_B2A = 2      # 2
F_B2B = 4      # 2
F_SXY = 6      # 2
F_WH2 = 8      # 2
F_AREA2 = 10
F_M = 11
F_MCLSV = 12
NF = 13

MAX_WAITS = 1


def _split_excess_waits(nc):
    """This neuronxcc build rejects >1 sem wait on several instruction
    classes; hoist extras onto same-engine Drain carriers placed before."""
    for f in nc.m.functions:
        for bb in f.blocks:
            new_list = []
            for ins in bb.instructions:
                si = ins.sync_info
                if si is not None and len(si.on_wait) > MAX_WAITS:
                    waits = list(si.on_wait)
                    excess, keep = waits[:-MAX_WAITS], waits[-MAX_WAITS:]
                    while excess:
                        chunk, excess = excess[:MAX_WAITS], excess[MAX_WAITS:]
                        carrier = mybir.InstDrain(
                            name=nc.get_next_instruction_name(),
                            engine=ins.engine, ins=[], outs=[],
                            bass_is_fusable=False,
                            sync_info=mybir.SyncInfo(on_wait=chunk, on_update=[]),
                        )
                        nc.register_instruction(carrier)
                        new_list.append(carrier)
                    si.on_wait = keep
                new_list.append(ins)
            bb.instructions[:] = new_list


def _V(tap, dims, extra_off=0):
    """Custom free-dim view of a tile AP (keeps the partition dim)."""
    return AP(tensor=tap.tensor, offset=tap.offset + extra_off,
              ap=[list(tap.ap[0])] + [list(d) for d in dims])


def _wrap16(idxs):
    n = idxs.shape[0]
    base16 = idxs.reshape(n // 16, 16).T.astype(np.int16)
    return np.tile(base16, (8, 1))


# ---------------- host preparation ----------------

def _prep(preds, targets):
    preds = np.asarray(preds, np.float32)
    targets = np.asarray(targets, np.float32)
    M = targets.shape[0]
    dt = np.float32

    scale = np.array([1, 1, W, H, W, H], dt)
    gt = (targets * scale).astype(dt)
    x0 = gt[:, 2].astype(np.int32)
    y0 = gt[:, 3].astype(np.int32)
    quad = np.array([[0, 0], [1, 0], [0, 1], [1, 1]], np.int32)
    gijx = x0[None, :] + quad[:, 0:1]
    gijy = y0[None, :] + quad[:, 1:2]
    m4 = (np.minimum(np.where(gijx < H, gijx, 0),
                     np.where(gijy < H, gijy, 0)) > 0)      # [4, M]
    b = targets[:, 0].astype(np.int32)
    gcls = targets[:, 1].astype(np.int32)

    gx, gy, gw, gh = gt[:, 2], gt[:, 3], gt[:, 4], gt[:, 5]
    half = dt(0.5)
    b2x1 = (gx - gw * half).astype(dt)
    b2x2 = (gx + gw * half).astype(dt)
    b2y1 = (gy - gh * half).astype(dt)
    b2y2 = (gy + gh * half).astype(dt)
    w2 = (b2x2 - b2x1).astype(dt)
    h2 = ((b2y2 - b2y1) + dt(1e-7)).astype(dt)
    area2h = (w2 * h2).astype(dt)
    sx2 = (b2x1 + b2x2).astype(dt)
    sy2 = (b2y1 + b2y2).astype(dt)

    cnt_m = max(int(m4.sum()), 1)
    r0 = (y0.astype(np.int64) * W + x0)
    core = b >> 1
    lbv_all = b & 1

    cnts = np.zeros((NCORES, 2), np.int64)
    for k in range(NCORES):
        cnts[k, 0] = int(((core == k) & (lbv_all == 0)).sum())
        cnts[k, 1] = int(((core == k) & (lbv_all == 1)).sum())
    J0 = int(max(1, math.ceil(cnts[:, 0].max() / 128)))
    J1 = int(max(1, math.ceil(cnts[:, 1].max() / 128)))
    Jr = J0 + J1
    G2 = 4 * Jr
    J2 = 2 * Jr

    per_core = []
    for k in range(NCORES):
        pc = preds[BPC * k:BPC * (k + 1)]
        reg = np.zeros((BPC, HW, 16), dt)
        t5 = pc[:, 0:5].reshape(BPC, 5, HW).transpose(0, 2, 1)
        reg[:, :, 0:5] = t5
        reg[:, :-W, 8:13] = t5[:, W:, :]
        clsarr = np.ascontiguousarray(pc[:, 5:85]).reshape(-1)
        pobjd = np.ascontiguousarray(pc[:, 0]).reshape(128, 400)

        hostf = np.zeros((128, NF, G2), dt)
        hostf[:, F_B2B:F_B2B + 2] = 1.0
        hostf[:, F_SXY:F_SXY + 2] = 1.0
        hostf[:, F_WH2:F_WH2 + 2] = 1.0
        hostf[:, F_AREA2] = 1.0
        oh4 = np.zeros((128, Jr, 4), dt)
        phic = np.full((128, J2), -1.0, dt)
        regg = np.zeros((2 * Jr, 128), np.int64)
        clsg0 = np.zeros((2 * J0, 128), np.int64)
        clsg1 = np.zeros((2 * J1, 128), np.int64)
        candcell = np.full((128, G2), -1, np.int64)
        candorig = np.full((128, G2), -1, np.int64)
        hostb = np.zeros((128, 3, G2), dt)
        crossing = []

        for lbv in (0, 1):
            gl = np.where((core == k) & (lbv_all == lbv))[0]
            joff = 0 if lbv == 0 else J0
            cg = clsg0 if lbv == 0 else clsg1
            for i, g in enumerate(gl):
                p = i % 128
                jrel = i // 128
                j = jrel + joff
                rr = int(r0[g])
                s = rr & 3
                bb0 = lbv * 6400 + (rr >> 2)
                regg[2 * j, p] = bb0
                regg[2 * j + 1, p] = min(bb0 + 1, REGROWS - 1)
                oh4[p, j, s] = 1.0
                for win in (0, 1):
                    yy = int(y0[g]) + win
                    if yy <= H - 1:
                        flat = int(gcls[g]) * HW + yy * W + int(x0[g])
                        cg[jrel * 2 + win, p] = flat >> 6
                        phic[p, j * 2 + win] = dt(flat & 63)
                for cell in (0, 1):
                    for win in (0, 1):
                        cw = cell * 2 + win
                        col = cw * Jr + j
                        q = win * 2 + cell
                        mm = bool(m4[q, g])
                        gi = (int(x0[g]) + cell) if mm else 0
                        gj = (int(y0[g]) + win) if mm else 0
                        hostf[p, F_GIJ + 0, col] = gi
                        hostf[p, F_GIJ + 1, col] = gj
                        hostf[p, F_M, col] = 1.0 if mm else 0.0
                        hostf[p, F_B2A + 0, col] = b2x1[g]
                        hostf[p, F_B2A + 1, col] = b2y1[g]
                        hostf[p, F_B2B + 0, col] = b2x2[g]
                        hostf[p, F_B2B + 1, col] = b2y2[g]
                        hostf[p, F_SXY + 0, col] = sx2[g]
                        hostf[p, F_SXY + 1, col] = sy2[g]
                        hostf[p, F_WH2 + 0, col] = w2[g]
                        hostf[p, F_WH2 + 1, col] = h2[g]
                        hostf[p, F_AREA2, col] = area2h[g]
                        hostf[p, F_MCLSV, col] = 1.0 if mm else 0.0
                        hostb[p, 0, col] = 1.0 if mm else 0.0
                        hostb[p, 1, col] = 1.0 - lbv
                        hostb[p, 2, col] = float(lbv)
                        candorig[p, col] = q * M + g
                        if mm:
                            candcell[p, col] = (int(b[g]) * HW + gj * W + gi)
                            if cell == 1:
                                yy = int(y0[g]) + win
                                flat = (int(gcls[g]) * HW + yy * W
                                        + int(x0[g]))
                                if (flat & 63) == 63:
                                    hostf[p, F_MCLSV, col] = 0.0
                                    pv = float(preds[BPC * k + lbv,
                                               5 + int(gcls[g]), yy,
                                               int(x0[g]) + 1])
                                    crossing.append((p, col, pv))

        idx16 = np.concatenate([
            _wrap16(regg.reshape(-1)),
            _wrap16(clsg0.reshape(-1)),
            _wrap16(clsg1.reshape(-1)),
        ], axis=1)

        # two-level class one-hots: hi = phi>>3 (8 wide), lo = phi&7 (8 wide)
        phii = phic.astype(np.int64)
        valid = phic >= 0
        ohhi = np.zeros((128, J2, 8), dt)
        ohlo = np.zeros((128, J2, 8), dt)
        pp, cc2 = np.where(valid)
        ohhi[pp, cc2, phii[pp, cc2] >> 3] = 1.0
        ohlo[pp, cc2, phii[pp, cc2] & 7] = 1.0

        big = np.concatenate([
            hostf.reshape(128, NF * G2),
            oh4.reshape(128, Jr * 4),
            ohhi.reshape(128, J2 * 8),
            ohlo.reshape(128, J2 * 8),
        ], axis=1)

        per_core.append(dict(
            regarr=reg.reshape(-1), clsarr=clsarr, pobjd=pobjd,
            idx16=idx16, big=big,
            hostb=hostb.reshape(128, 3, G2),
            candcell=candcell, candorig=candorig, crossing=crossing,
        ))

    meta = dict(J0=J0, J1=J1, Jr=Jr, G2=G2, J2=J2, cnt_m=cnt_m, M=M)
    return per_core, meta


# ---------------- phase A program ----------------

def _build_phase_a(meta):
    J0, J1 = meta["J0"], meta["J1"]
    Jr, G2, J2 = meta["Jr"], meta["G2"], meta["J2"]
    KR = 2 * Jr * 128
    K0 = 2 * J0 * 128
    K1 = 2 * J1 * 128
    KTW = (KR + K0 + K1) // 16
    OH4 = NF * G2
    OHHI = OH4 + Jr * 4
    OHLO = OHHI + J2 * 8
    BIGW = OHLO + J2 * 8
    AOUT = 3 * G2 + 8

    nc = bass.Bass("TRN2", debug=False, num_swdge_queues=4)
    regT = nc.dram_tensor("regarr", [BPC * HW * 16], f32, kind="ExternalInput")
    clsT = nc.dram_tensor("clsarr", [BPC * 80 * HW], f32, kind="ExternalInput")
    idxT = nc.dram_tensor("idx16", [128, KTW], i16, kind="ExternalInput")
    bigT = nc.dram_tensor("big", [128, BIGW], f32, kind="ExternalInput")
    aoutT = nc.dram_tensor("aout", [128, AOUT], f32, kind="ExternalOutput")

    with TileContext(nc) as tc:
        with tc.tile_pool(name="sbuf", bufs=1) as pool:
            nc.gpsimd.load_library(library_config.mlp)

            idx_t = pool.tile([128, KTW], i16)
            nc.sync.dma_start(out=idx_t[:], in_=idxT.ap())
            big = pool.tile([128, BIGW], f32)
            nc.sync.dma_start(out=big[:], in_=bigT.ap())
            hf = big[:, 0:NF * G2].rearrange("p (f g) -> p f g", f=NF)

            out_t = pool.tile([128, AOUT], f32)
            nc.vector.memset(out_t[:], 0.0)

            # ---- gathers: reg first (feeds extraction + math) ----
            gt_reg = pool.tile([128, 2 * Jr, 64], f32)
            nc.gpsimd.dma_gather(
                out_ap=gt_reg[:],
                in_ap=regT.ap().rearrange("(r e) -> r e", e=64),
                idxs_ap=idx_t[:, 0:KR // 16],
                num_idxs=KR, num_idxs_reg=KR, elem_size=64,
                single_packet=False, queue_num=0)
            gt_cls = pool.tile([128, J2 * 64 + 8], f32)
            nc.gpsimd.dma_gather(
                out_ap=gt_cls[:, 0:2 * J0 * 64].rearrange(
                    "p (a b) -> p a b", b=64),
                in_ap=clsT.ap()[0:80 * HW].rearrange("(r e) -> r e", e=64),
                idxs_ap=idx_t[:, KR // 16:(KR + K0) // 16],
                num_idxs=K0, num_idxs_reg=K0, elem_size=64,
                single_packet=False, queue_num=1)
            nc.gpsimd.dma_gather(
                out_ap=gt_cls[:, 2 * J0 * 64:J2 * 64].rearrange(
                    "p (a b) -> p a b", b=64),
                in_ap=clsT.ap()[80 * HW:].rearrange("(r e) -> r e", e=64),
                idxs_ap=idx_t[:, (KR + K0) // 16:(KR + K0 + K1) // 16],
                num_idxs=K1, num_idxs_reg=K1, elem_size=64,
                single_packet=False, queue_num=2)

            tt = nc.vector.tensor_tensor
            ts = nc.vector.tensor_scalar
            stt = nc.vector.scalar_tensor_tensor
            act = nc.scalar.activation

            def T(shape, tag):
                return pool.tile([128] + shape, f32, name=tag, tag=tag)

            def hfv(i, n=1):
                if n == 1:
                    return hf[:, i, :]
                return hf[:, i:i + n, :]

            # ---- reg extraction: 4-wide one-hot per (cell, win) ----
            ext = T([4, Jr, 5], "ext")
            grap = gt_reg[:].rearrange("p a b -> p (a b)")
            ohv = _V(big[:, OH4:OH4 + Jr * 4], [[4, Jr], [0, 5], [1, 4]])
            for cw in range(4):
                cell, win = cw >> 1, cw & 1
                gv = _V(grap, [[128, Jr], [1, 5], [16, 4]],
                        extra_off=cell * 16 + win * 8)
                prod = T([Jr, 5, 4], f"prodr{cw}")
                tt(out=prod[:], in0=gv, in1=ohv, op=Alu.mult)
                nc.vector.tensor_reduce(out=ext[:, cw], in_=prod[:],
                                        axis=X, op=Alu.add)

            eap = ext[:].rearrange("p a b c -> p (a b c)")
            pobj_v = _V(eap, [[5 * Jr, 4], [5, Jr]], extra_off=0)
            pr01_v = _V(eap, [[1, 2], [5 * Jr, 4], [5, Jr]], extra_off=1)
            pr23_v = _V(eap, [[1, 2], [5 * Jr, 4], [5, Jr]], extra_off=3)

            # pobj for phase B (fills the tanh/sigmoid latency)
            nc.vector.tensor_copy(
                out=out_t[:, 2 * G2:3 * G2].rearrange(
                    "p (a b) -> p a b", b=Jr),
                in_=pobj_v)

            def r4(apx):   # [128, 2, G2] -> [128, 2, 4, Jr]
                return apx.rearrange("p c (a b) -> p c a b", b=Jr)

            # ---- SIoU math (manually scheduled for ACT overlap) ----
            t01 = T([2, G2], "t01")
            act(r4(t01[:]), pr01_v, Act.Tanh)
            sg = T([2, G2], "sg")
            act(r4(sg[:]), pr23_v, Act.Sigmoid)

            txy = T([2, G2], "txy")
            tt(out=txy[:], in0=t01[:], in1=hfv(F_GIJ, 2), op=Alu.add)
            b1a = T([2, G2], "b1a")
            stt(out=b1a[:], in0=sg[:], scalar=-80.0, in1=txy[:],
                op0=Alu.mult, op1=Alu.add)
            b1b = T([2, G2], "b1b")
            stt(out=b1b[:], in0=sg[:], scalar=80.0, in1=txy[:],
                op0=Alu.mult, op1=Alu.add)
            wh1 = T([2, G2], "wh1")
            tt(out=wh1[:], in0=b1b[:], in1=b1a[:], op=Alu.subtract)
            s2 = T([2, G2], "s2")
            tt(out=s2[:], in0=hfv(F_SXY, 2), in1=b1a[:], op=Alu.subtract)
            tt(out=s2[:], in0=s2[:], in1=b1b[:], op=Alu.subtract)
            # issue ACT early: square/abs of s2 overlap the iou0 path below
            sq = T([2, G2], "sq")
            act(sq[:], s2[:], Act.Square)
            sabs = T([2, G2], "sabs")
            act(sabs[:], s2[:], Act.Abs)

            b2a = hfv(F_B2A, 2)
            b2b = hfv(F_B2B, 2)
            mn = T([2, G2], "mn")
            tt(out=mn[:], in0=b1b[:], in1=b2b, op=Alu.min)
            mx = T([2, G2], "mx")
            tt(out=mx[:], in0=b1a[:], in1=b2a, op=Alu.max)
            dcl = T([2, G2], "dcl")
            tt(out=dcl[:], in0=mn[:], in1=mx[:], op=Alu.subtract)
            ts(dcl[:], dcl[:], 0.0, None, Alu.max)
            inter = T([G2], "inter")
            tt(out=inter[:], in0=dcl[:, 0, :], in1=dcl[:, 1, :], op=Alu.mult)
            area1 = T([G2], "area1")
            tt(out=area1[:], in0=wh1[:, 0, :], in1=wh1[:, 1, :], op=Alu.mult)
            u = T([G2], "u")
            stt(out=u[:], in0=inter[:], scalar=-1.0, in1=area1[:],
                op0=Alu.mult, op1=Alu.add)
            tt(out=u[:], in0=u[:], in1=hfv(F_AREA2), op=Alu.add)
            invu = T([G2], "invu")
            nc.vector.reciprocal(invu[:], u[:])
            iou0 = T([G2], "iou0")
            tt(out=iou0[:], in0=inter[:], in1=invu[:], op=Alu.mult)

            mx2 = T([2, G2], "mx2")
            tt(out=mx2[:], in0=b1b[:], in1=b2b, op=Alu.max)
            mn2 = T([2, G2], "mn2")
            tt(out=mn2[:], in0=b1a[:], in1=b2a, op=Alu.min)
            cwh = T([2, G2], "cwh")
            tt(out=cwh[:], in0=mx2[:], in1=mn2[:], op=Alu.subtract)
            invcw = T([2, G2], "invcw")
            nc.vector.reciprocal(invcw[:], cwh[:])
            rr0 = T([2, G2], "rr0")
            tt(out=rr0[:], in0=s2[:], in1=invcw[:], op=Alu.mult)
            gr = T([2, G2], "gr")
            tt(out=gr[:], in0=rr0[:], in1=rr0[:], op=Alu.mult)

            wh2t = hfv(F_WH2, 2)
            dwh = T([2, G2], "dwh")
            tt(out=dwh[:], in0=wh1[:], in1=wh2t, op=Alu.subtract)
            adwh = T([2, G2], "adwh")
            stt(out=adwh[:], in0=dwh[:], scalar=-1.0, in1=dwh[:],
                op0=Alu.mult, op1=Alu.max)
            mxw = T([2, G2], "mxw")
            tt(out=mxw[:], in0=wh1[:], in1=wh2t, op=Alu.max)
            nc.vector.reciprocal(mxw[:], mxw[:])
            omw = T([2, G2], "omw")
            tt(out=omw[:], in0=adwh[:], in1=mxw[:], op=Alu.mult)
            ewh = T([2, G2], "ewh")
            act(ewh[:], omw[:], Act.Exp, scale=-1.0)

            ssum = T([G2], "ssum")
            tt(out=ssum[:], in0=sq[:, 0, :], in1=sq[:, 1, :], op=Alu.add)
            rs = T([G2], "rs")
            nc.vector.reciprocal(rs[:], ssum[:])
            invsig = T([G2], "invsig")
            act(invsig[:], rs[:], Act.Sqrt)

            oe = T([2, G2], "oe")
            ts(oe[:], ewh[:], -1.0, 1.0, Alu.mult, Alu.add)
            tt(out=oe[:], in0=oe[:], in1=oe[:], op=Alu.mult)
            tt(out=oe[:], in0=oe[:], in1=oe[:], op=Alu.mult)
            shp = T([G2], "shp")
            tt(out=shp[:], in0=oe[:, 0, :], in1=oe[:, 1, :], op=Alu.add)

            sin12 = T([2, G2], "sin12")
            tt(out=sin12[:], in0=sabs[:],
               in1=_V(invsig[:], [[0, 2], [1, G2]]), op=Alu.mult)
            sina = T([G2], "sina")
            tt(out=sina[:], in0=sin12[:, 0, :], in1=sin12[:, 1, :], op=Alu.min)
            sa2 = T([G2], "sa2")
            tt(out=sa2[:], in0=sina[:], in1=sina[:], op=Alu.mult)
            om = T([G2], "om")
            ts(om[:], sa2[:], -1.0, 1.0, Alu.mult, Alu.add)
            rt = T([G2], "rt")
            act(rt[:], om[:], Act.Sqrt)
            gam4 = T([G2], "gam4")
            tt(out=gam4[:], in0=sina[:], in1=rt[:], op=Alu.mult)
            ts(gam4[:], gam4[:], 0.5, -0.5, Alu.mult, Alu.add)
            tt(out=gr[:], in0=gr[:], in1=_V(gam4[:], [[0, 2], [1, G2]]),
               op=Alu.mult)
            eg = T([2, G2], "eg")
            act(eg[:], gr[:], Act.Exp)
            t_eg = T([G2], "t_eg")
            tt(out=t_eg[:], in0=eg[:, 0, :], in1=eg[:, 1, :], op=Alu.add)
            c1 = T([G2], "c1")
            stt(out=c1[:], in0=shp[:], scalar=-1.0, in1=t_eg[:],
                op0=Alu.mult, op1=Alu.add)
            ts(c1[:], c1[:], 0.5, -1.0, Alu.mult, Alu.add)
            iou_v = out_t[:, 0:G2]
            tt(out=iou_v, in0=iou0[:], in1=c1[:], op=Alu.add)

            # sum(iou*m) partial per partition
            scr = T([G2], "scr")
            stt(out=scr[:], in0=iou_v, scalar=1.0, in1=hfv(F_M),
                op0=Alu.mult, op1=Alu.mult,
                accum_out=out_t[:, 3 * G2:3 * G2 + 1])

            # ---- class extraction: two-level one-hot (hi 8 x lo 8) ----
            strip = T([J2, 9], "strip")
            prod1 = T([J2, 9, 8], "prod1")
            gv1 = _V(gt_cls[:], [[64, J2], [1, 9], [8, 8]])
            ohhiv = _V(big[:, OHHI:OHHI + J2 * 8], [[8, J2], [0, 9], [1, 8]])
            tt(out=prod1[:], in0=gv1, in1=ohhiv, op=Alu.mult)
            nc.vector.tensor_reduce(out=strip[:], in_=prod1[:],
                                    axis=X, op=Alu.add)
            ohlov = big[:, OHLO:OHLO + J2 * 8].rearrange(
                "p (a b) -> p a b", b=8)
            pg = T([2, J2], "pg")
            prod2 = T([J2, 8], "prod2")
            tt(out=prod2[:], in0=strip[:, :, 0:8], in1=ohlov, op=Alu.mult)
            nc.vector.tensor_reduce(out=pg[:, 0], in_=prod2[:],
                                    axis=X, op=Alu.add)
            prod3 = T([J2, 8], "prod3")
            tt(out=prod3[:], in0=_V(strip[:].rearrange("p a b -> p (a b)"),
                                    [[9, J2], [1, 8]], extra_off=1),
               in1=ohlov, op=Alu.mult)
            nc.vector.tensor_reduce(out=pg[:, 1], in_=prod3[:],
                                    axis=X, op=Alu.add)
            ts(pg[:], pg[:], 1e-38, None, Alu.max)
            lnt = T([2, J2], "lnt")
            act(lnt[:], pg[:], Act.Ln)
            lnp_in = _V(lnt[:].rearrange("p a b -> p (a b)"),
                        [[J2, 2], [1, 2], [2, Jr]])
            tt(out=out_t[:, G2:2 * G2].rearrange(
                   "p (c w j) -> p c w j", c=2, w=2),
               in0=lnp_in,
               in1=hfv(F_MCLSV).rearrange("p (c w j) -> p c w j", c=2, w=2),
               op=Alu.mult)

            nc.sync.dma_start(out=aoutT.ap(), in_=out_t[:])

    lower_extended_insts(nc)
    _split_excess_waits(nc)
    return nc


# ---------------- phase B program ----------------

def _build_phase_b(meta):
    G2 = meta["G2"]
    AOUT = 3 * G2 + 8
    # merged input: [aout(iou host-masked) | fv/2 | imean | pobjd]
    BINW = AOUT + G2 + 1 + 400

    nc = bass.Bass("TRN2", debug=False)
    binT = nc.dram_tensor("binall", [128, BINW], f32, kind="ExternalInput")
    boutT = nc.dram_tensor("bout", [128, 8], f32, kind="ExternalOutput")

    with TileContext(nc) as tc:
        with tc.tile_pool(name="sbuf", bufs=1) as pool:
            bi = pool.tile([128, BINW], f32)
            nc.sync.dma_start(out=bi[:], in_=binT.ap())

            ob = pool.tile([128, 8], f32)
            nc.vector.memset(ob[:], 0.0)

            iou_v = bi[:, 0:G2]
            lnp_v = bi[:, G2:2 * G2]
            pox = bi[:, 2 * G2:3 * G2]
            fv = bi[:, AOUT:AOUT + G2]
            im = bi[:, AOUT + G2:AOUT + G2 + 1]
            pod = bi[:, AOUT + G2 + 1:BINW]

            tt = nc.vector.tensor_tensor
            ts = nc.vector.tensor_scalar
            stt = nc.vector.scalar_tensor_tensor

            def T(shape, tag):
                return pool.tile([128] + shape, f32, name=tag, tag=tag)

            f_v = T([G2], "f")
            tt(out=f_v[:], in0=iou_v,
               in1=im.to_broadcast([128, G2]), op=Alu.is_gt)

            # bout0 = sum f*iou ; bout1 = sum f*lnp ; bout2 = sum f
            s0 = T([G2], "s0")
            stt(out=s0[:], in0=iou_v, scalar=1.0, in1=f_v[:],
                op0=Alu.mult, op1=Alu.mult, accum_out=ob[:, 0:1])
            s1 = T([G2], "s1")
            stt(out=s1[:], in0=lnp_v, scalar=1.0, in1=f_v[:],
                op0=Alu.mult, op1=Alu.mult, accum_out=ob[:, 1:2])
            nc.vector.tensor_reduce(out=ob[:, 2:3], in_=f_v[:],
                                    axis=X, op=Alu.add)

            # obj corr: f*(sl1(pobj-iou)*fval - 0.375*pobj^2)
            # sl1 = 0.5*mm*(2*ad - mm), mm = min(ad,1); the 0.5 is folded
            # into fv (host ships fval/2)
            d = T([G2], "d")
            tt(out=d[:], in0=pox, in1=iou_v, op=Alu.subtract)
            ad = T([G2], "ad")
            stt(out=ad[:], in0=d[:], scalar=-1.0, in1=d[:],
                op0=Alu.mult, op1=Alu.max)
            mm_ = T([G2], "mm_")
            ts(mm_[:], ad[:], 1.0, None, Alu.min)
            t2 = T([G2], "t2")
            stt(out=t2[:], in0=ad[:], scalar=2.0, in1=mm_[:],
                op0=Alu.mult, op1=Alu.subtract)
            tt(out=t2[:], in0=t2[:], in1=mm_[:], op=Alu.mult)
            tt(out=t2[:], in0=t2[:], in1=fv, op=Alu.mult)
            po2 = T([G2], "po2")
            stt(out=po2[:], in0=pox, scalar=-0.375, in1=pox,
                op0=Alu.mult, op1=Alu.mult)
            tt(out=t2[:], in0=t2[:], in1=po2[:], op=Alu.add)
            s4 = T([G2], "s4")
            stt(out=s4[:], in0=t2[:], scalar=1.0, in1=f_v[:],
                op0=Alu.mult, op1=Alu.mult, accum_out=ob[:, 3:4])

            # dense obj baseline partial: sum(pobj^2)
            s5 = pool.tile([128, 400], f32)
            stt(out=s5[:], in0=pod, scalar=1.0, in1=pod,
                op0=Alu.mult, op1=Alu.mult, accum_out=ob[:, 4:5])

            nc.sync.dma_start(out=boutT.ap(), in_=ob[:])

    lower_extended_insts(nc)
    _split_excess_waits(nc)
    return nc


# ---------------- host-side patches ----------------

def _sl1(x):
    ax = abs(x)
    return 0.5 * x * x if ax < 1.0 else ax - 0.5


def _host_patches(per_core, meta, aouts, iou_mean):
    """Returns (corr_patch, s2_patch): corr_patch is subtracted from the
    device obj-corr sum (collision losers); s2_patch is added to the
    device sum f*lnp (class phi=63 crossings)."""
    G2 = meta["G2"]
    corr_patch = 0.0
    s2_patch = 0.0
    for k, d in enumerate(per_core):
        a = aouts[k]
        iou = a[:, 0:G2]
        pobj = a[:, 2 * G2:3 * G2]
        hb = d["hostb"].reshape(128, 3, G2)
        m = hb[:, 0, :]
        mkB = hb[:, 2, :]
        f = (iou > iou_mean) & (m > 0)

        # nperb for this core's two batches (exact integer counts)
        npA = max(float((f & (mkB < 0.5)).sum()), 0.5)
        npB = max(float((f & (mkB > 0.5)).sum()), 0.5)
        fvalA = 6400.0 / npA
        fvalB = 6400.0 / npB

        # collision dedup: group f-positive candidates by cell id
        cells = d["candcell"]
        fpos = f & (cells >= 0)
        if fpos.any():
            cid = cells[fpos]
            orig = d["candorig"][fpos]
            iouv = iou[fpos]
            pov = pobj[fpos]
            isB = mkB[fpos] > 0.5
            order = np.argsort(cid, kind="stable")
            cid, orig, iouv, pov, isB = (cid[order], orig[order],
                                         iouv[order], pov[order], isB[order])
            i = 0
            n = len(cid)
            while i < n:
                jx = i
                while jx + 1 < n and cid[jx + 1] == cid[i]:
                    jx += 1
                if jx > i:
                    widx = i + int(np.argmax(orig[i:jx + 1]))
                    for t in range(i, jx + 1):
                        if t == widx:
                            continue
                        fval = fvalB if isB[t] else fvalA
                        corr_patch += (_sl1(float(pov[t]) - float(iouv[t]))
                                       * fval - 0.375 * float(pov[t]) ** 2)
                i = jx + 1

        # class crossing patch
        for (p, col, pv) in d["crossing"]:
            if f[p, col]:
                s2_patch += math.log(max(pv, 1e-38))
    return corr_patch, s2_patch


# ---------------- main entry ----------------

_CACHE = {}


def kernel(preds, targets):
    per_core, meta = _prep(preds, targets)

    key = (meta["J0"], meta["J1"])
    if key not in _CACHE:
        _CACHE[key] = (_build_phase_a(meta), _build_phase_b(meta))
    nc_a, nc_b = _CACHE[key]

    core_ids = list(range(NCORES))
    in_maps_a = [dict(regarr=d["regarr"], clsarr=d["clsarr"],
                      idx16=d["idx16"], big=d["big"]) for d in per_core]
    res_a = run_bass_kernel_spmd(nc_a, in_maps_a, core_ids)

    G2 = meta["G2"]
    aouts = [res_a.results[k]["aout"] for k in core_ids]
    sum_im = sum(float(a[:, 3 * G2].sum(dtype=np.float64)) for a in aouts)
    iou_mean = np.float32(np.float32(sum_im) / np.float32(meta["cnt_m"]))

    imean_arr = np.full((128, 1), iou_mean, np.float32)
    in_maps_b = []
    for k in core_ids:
        d = per_core[k]
        a = aouts[k]
        hb = d["hostb"]
        m_h, mkA, mkB = hb[:, 0], hb[:, 1], hb[:, 2]
        fh = (a[:, 0:G2] > iou_mean) & (m_h > 0)
        npA = max(float((fh & (mkA > 0.5)).sum()), 0.5)
        npB = max(float((fh & (mkB > 0.5)).sum()), 0.5)
        fv2 = (mkA * np.float32(3200.0 / npA)
               + mkB * np.float32(3200.0 / npB)).astype(np.float32)
        d["fvals"] = (6400.0 / npA, 6400.0 / npB)
        am = a.copy()
        am[:, 0:G2] = np.where(m_h > 0, a[:, 0:G2], np.float32(-1e4))
        binall = np.concatenate(
            [am, fv2, imean_arr, d["pobjd"]], axis=1).astype(np.float32)
        in_maps_b.append(dict(binall=binall))
    res_b = run_bass_kernel_spmd(nc_b, in_maps_b, core_ids)

    bouts = [res_b.results[k]["bout"] for k in core_ids]
    Sfi = sum(float(o[:, 0].sum(dtype=np.float64)) for o in bouts)
    S2 = sum(float(o[:, 1].sum(dtype=np.float64)) for o in bouts)
    cnt_f = max(sum(float(o[:, 2].sum(dtype=np.float64)) for o in bouts), 1.0)
    corr = sum(float(o[:, 3].sum(dtype=np.float64)) for o in bouts)
    base = sum(float(o[:, 4].sum(dtype=np.float64)) for o in bouts)

    corr_patch, s2_patch = _host_patches(per_core, meta, aouts, iou_mean)

    iou_loss = np.float32((cnt_f - Sfi) / cnt_f)
    cls_loss = np.float32(-(S2 + s2_patch) / cnt_f)
    obj_loss = np.float32((0.375 * base + corr - corr_patch) / (N * HW))
    loss = np.float32(iou_loss * 8 + obj_loss * 16 + cls_loss)
    return (iou_loss, obj_loss, cls_loss, loss)


# revision 14
# speedup vs baseline: 2.1160x; 1.0491x over previous
"""Trainium2 Bass kernel for nn_DetectorLoss (SIoU detector loss).

Strategy: data-parallel over batch N=16 -> 8 cores x 2 batches.

Host re-lays preds (input-independent permutations only):
  - regarr: per cell r a 16-float record [ch0..4 @ r | pad | ch0..4 @ r+160 | pad]
    so ONE 256B-aligned dma_gather descriptor pair covers all 4 quadrant
    candidates' obj+reg channels of a ground truth (window of 30 floats at
    16*r0, phase in {0,16,32,48} -> 4-wide one-hot extraction).
  - clsarr: plain [80, HW] class channels per batch; one 64-float row per
    (GT, y-row) covers both x cells; 64-wide one-hot extraction.

Phase A computes per-candidate SIoU iou, log-class prob, pobj and the
partial sum(iou*m); host combines the global iou_mean; phase B applies the
f-mask, computes the masked reductions and the dense obj baseline.
Cell-collision dedup (rare) and phi=63 class-row crossings (rare) are
patched exactly on host from the per-candidate outputs.
"""

import math
import numpy as np

import concourse.bass as bass
import concourse.mybir as mybir
from concourse import library_config
from concourse.bass import AP
from concourse.library_overlay import lower_extended_insts
from concourse.tile import TileContext
from concourse.bass_utils import run_bass_kernel_spmd

# ---------------- problem constants (hardcoded per spec) ----------------
N, C, H, W = 16, 85, 160, 160
HW = H * W                  # 25600
NCORES = 8
BPC = 2
M_DEFAULT = 4096

f32 = mybir.dt.float32
i16 = mybir.dt.int16
Alu = mybir.AluOpType
Act = mybir.ActivationFunctionType
X = mybir.AxisListType.X

REGROWS = BPC * HW * 16 // 64      # 12800
CLSROWS = 80 * HW // 64            # 32000 per batch

# hostf field indices
F_GIJ = 0      # 2
F_B2A = 2      # 2
F_B2B = 4      # 2
F_SXY = 6      # 2
F_WH2 = 8      # 2
F_AREA2 = 10
F_M = 11
F_MCLSV = 12
NF = 13

MAX_WAITS = 1


def _split_excess_waits(nc):
    """This neuronxcc build rejects >1 sem wait on several instruction
    classes; hoist extras onto same-engine Drain carriers placed before."""
    for f in nc.m.functions:
        for bb in f.blocks:
            new_list = []
            for ins in bb.instructions:
                si = ins.sync_info
                if si is not None and len(si.on_wait) > MAX_WAITS:
                    waits = list(si.on_wait)
                    excess, keep = waits[:-MAX_WAITS], waits[-MAX_WAITS:]
                    while excess:
                        chunk, excess = excess[:MAX_WAITS], excess[MAX_WAITS:]
                        carrier = mybir.InstDrain(
                            name=nc.get_next_instruction_name(),
                            engine=ins.engine, ins=[], outs=[],
                            bass_is_fusable=False,
                            sync_info=mybir.SyncInfo(on_wait=chunk, on_update=[]),
                        )
                        nc.register_instruction(carrier)
                        new_list.append(carrier)
                    si.on_wait = keep
                new_list.append(ins)
            bb.instructions[:] = new_list


def _V(tap, dims, extra_off=0):
    """Custom free-dim view of a tile AP (keeps the partition dim)."""
    return AP(tensor=tap.tensor, offset=tap.offset + extra_off,
              ap=[list(tap.ap[0])] + [list(d) for d in dims])


def _wrap16(idxs):
    n = idxs.shape[0]
    base16 = idxs.reshape(n // 16, 16).T.astype(np.int16)
    return np.tile(base16, (8, 1))


# ---------------- host preparation ----------------

def _prep(preds, targets):
    preds = np.asarray(preds, np.float32)
    targets = np.asarray(targets, np.float32)
    M = targets.shape[0]
    dt = np.float32

    scale = np.array([1, 1, W, H, W, H], dt)
    gt = (targets * scale).astype(dt)
    x0 = gt[:, 2].astype(np.int32)
    y0 = gt[:, 3].astype(np.int32)
    quad = np.array([[0, 0], [1, 0], [0, 1], [1, 1]], np.int32)
    gijx = x0[None, :] + quad[:, 0:1]
    gijy = y0[None, :] + quad[:, 1:2]
    m4 = (np.minimum(np.where(gijx < H, gijx, 0),
                     np.where(gijy < H, gijy, 0)) > 0)      # [4, M]
    b = targets[:, 0].astype(np.int32)
    gcls = targets[:, 1].astype(np.int32)

    gx, gy, gw, gh = gt[:, 2], gt[:, 3], gt[:, 4], gt[:, 5]
    half = dt(0.5)
    b2x1 = (gx - gw * half).astype(dt)
    b2x2 = (gx + gw * half).astype(dt)
    b2y1 = (gy - gh * half).astype(dt)
    b2y2 = (gy + gh * half).astype(dt)
    w2 = (b2x2 - b2x1).astype(dt)
    h2 = ((b2y2 - b2y1) + dt(1e-7)).astype(dt)
    area2h = (w2 * h2).astype(dt)
    sx2 = (b2x1 + b2x2).astype(dt)
    sy2 = (b2y1 + b2y2).astype(dt)

    cnt_m = max(int(m4.sum()), 1)
    r0 = (y0.astype(np.int64) * W + x0)
    core = b >> 1
    lbv_all = b & 1

    cnts = np.zeros((NCORES, 2), np.int64)
    for k in range(NCORES):
        cnts[k, 0] = int(((core == k) & (lbv_all == 0)).sum())
        cnts[k, 1] = int(((core == k) & (lbv_all == 1)).sum())
    J0 = int(max(1, math.ceil(cnts[:, 0].max() / 128)))
    J1 = int(max(1, math.ceil(cnts[:, 1].max() / 128)))
    Jr = J0 + J1
    G2 = 4 * Jr
    J2 = 2 * Jr

    per_core = []
    for k in range(NCORES):
        pc = preds[BPC * k:BPC * (k + 1)]
        reg = np.zeros((BPC, HW, 16), dt)
        t5 = pc[:, 0:5].reshape(BPC, 5, HW).transpose(0, 2, 1)
        reg[:, :, 0:5] = t5
        reg[:, :-W, 8:13] = t5[:, W:, :]
        clsarr = np.ascontiguousarray(pc[:, 5:85]).reshape(-1)
        pobjd = np.ascontiguousarray(pc[:, 0]).reshape(128, 400)

        hostf = np.zeros((128, NF, G2), dt)
        hostf[:, F_B2B:F_B2B + 2] = 1.0
        hostf[:, F_SXY:F_SXY + 2] = 1.0
        hostf[:, F_WH2:F_WH2 + 2] = 1.0
        hostf[:, F_AREA2] = 1.0
        oh4 = np.zeros((128, Jr, 4), dt)
        phic = np.full((128, J2), -1.0, dt)
        regg = np.zeros((2 * Jr, 128), np.int64)
        clsg0 = np.zeros((2 * J0, 128), np.int64)
        clsg1 = np.zeros((2 * J1, 128), np.int64)
        candcell = np.full((128, G2), -1, np.int64)
        candorig = np.full((128, G2), -1, np.int64)
        hostb = np.zeros((128, 3, G2), dt)
        crossing = []

        for lbv in (0, 1):
            gl = np.where((core == k) & (lbv_all == lbv))[0]
            joff = 0 if lbv == 0 else J0
            cg = clsg0 if lbv == 0 else clsg1
            for i, g in enumerate(gl):
                p = i % 128
                jrel = i // 128
                j = jrel + joff
                rr = int(r0[g])
                s = rr & 3
                bb0 = lbv * 6400 + (rr >> 2)
                regg[2 * j, p] = bb0
                regg[2 * j + 1, p] = min(bb0 + 1, REGROWS - 1)
                oh4[p, j, s] = 1.0
                for win in (0, 1):
                    yy = int(y0[g]) + win
                    if yy <= H - 1:
                        flat = int(gcls[g]) * HW + yy * W + int(x0[g])
                        cg[jrel * 2 + win, p] = flat >> 6
                        phic[p, j * 2 + win] = dt(flat & 63)
                for cell in (0, 1):
                    for win in (0, 1):
                        cw = cell * 2 + win
                        col = cw * Jr + j
                        q = win * 2 + cell
                        mm = bool(m4[q, g])
                        gi = (int(x0[g]) + cell) if mm else 0
                        gj = (int(y0[g]) + win) if mm else 0
                        hostf[p, F_GIJ + 0, col] = gi
                        hostf[p, F_GIJ + 1, col] = gj
                        hostf[p, F_M, col] = 1.0 if mm else 0.0
                        hostf[p, F_B2A + 0, col] = b2x1[g]
                        hostf[p, F_B2A + 1, col] = b2y1[g]
                        hostf[p, F_B2B + 0, col] = b2x2[g]
                        hostf[p, F_B2B + 1, col] = b2y2[g]
                        hostf[p, F_SXY + 0, col] = sx2[g]
                        hostf[p, F_SXY + 1, col] = sy2[g]
                        hostf[p, F_WH2 + 0, col] = w2[g]
                        hostf[p, F_WH2 + 1, col] = h2[g]
                        hostf[p, F_AREA2, col] = area2h[g]
                        hostf[p, F_MCLSV, col] = 1.0 if mm else 0.0
                        hostb[p, 0, col] = 1.0 if mm else 0.0
                        hostb[p, 1, col] = 1.0 - lbv
                        hostb[p, 2, col] = float(lbv)
                        candorig[p, col] = q * M + g
                        if mm:
                            candcell[p, col] = (int(b[g]) * HW + gj * W + gi)
                            if cell == 1:
                                yy = int(y0[g]) + win
                                flat = (int(gcls[g]) * HW + yy * W
                                        + int(x0[g]))
                                if (flat & 63) == 63:
                                    hostf[p, F_MCLSV, col] = 0.0
                                    pv = float(preds[BPC * k + lbv,
                                               5 + int(gcls[g]), yy,
                                               int(x0[g]) + 1])
                                    crossing.append((p, col, pv))

        idx16 = np.concatenate([
            _wrap16(regg.reshape(-1)),
            _wrap16(clsg0.reshape(-1)),
            _wrap16(clsg1.reshape(-1)),
        ], axis=1)

        # two-level class one-hots: hi = phi>>3 (8 wide), lo = phi&7 (8 wide)
        phii = phic.astype(np.int64)
        valid = phic >= 0
        ohhi = np.zeros((128, J2, 8), dt)
        ohlo = np.zeros((128, J2, 8), dt)
        pp, cc2 = np.where(valid)
        ohhi[pp, cc2, phii[pp, cc2] >> 3] = 1.0
        ohlo[pp, cc2, phii[pp, cc2] & 7] = 1.0

        big = np.concatenate([
            hostf.reshape(128, NF * G2),
            oh4.reshape(128, Jr * 4),
            ohhi.reshape(128, J2 * 8),
            ohlo.reshape(128, J2 * 8),
        ], axis=1)

        per_core.append(dict(
            regarr=reg.reshape(-1), clsarr=clsarr, pobjd=pobjd,
            idx16=idx16, big=big,
            hostb=hostb.reshape(128, 3, G2),
            candcell=candcell, candorig=candorig, crossing=crossing,
        ))

    meta = dict(J0=J0, J1=J1, Jr=Jr, G2=G2, J2=J2, cnt_m=cnt_m, M=M)
    return per_core, meta


# ---------------- phase A program ----------------

def _build_phase_a(meta):
    J0, J1 = meta["J0"], meta["J1"]
    Jr, G2, J2 = meta["Jr"], meta["G2"], meta["J2"]
    KR = 2 * Jr * 128
    K0 = 2 * J0 * 128
    K1 = 2 * J1 * 128
    KTW = (KR + K0 + K1) // 16
    OH4 = NF * G2
    OHHI = OH4 + Jr * 4
    OHLO = OHHI + J2 * 8
    BIGW = OHLO + J2 * 8
    AOUT = 3 * G2 + 8

    nc = bass.Bass("TRN2", debug=False, num_swdge_queues=4)
    regT = nc.dram_tensor("regarr", [BPC * HW * 16], f32, kind="ExternalInput")
    clsT = nc.dram_tensor("clsarr", [BPC * 80 * HW], f32, kind="ExternalInput")
    idxT = nc.dram_tensor("idx16", [128, KTW], i16, kind="ExternalInput")
    bigT = nc.dram_tensor("big", [128, BIGW], f32, kind="ExternalInput")
    aoutT = nc.dram_tensor("aout", [128, AOUT], f32, kind="ExternalOutput")

    with TileContext(nc) as tc:
        with tc.tile_pool(name="sbuf", bufs=1) as pool:
            nc.gpsimd.load_library(library_config.mlp)

            idx_t = pool.tile([128, KTW], i16)
            nc.sync.dma_start(out=idx_t[:], in_=idxT.ap())
            big = pool.tile([128, BIGW], f32)
            nc.sync.dma_start(out=big[:], in_=bigT.ap())
            hf = big[:, 0:NF * G2].rearrange("p (f g) -> p f g", f=NF)

            out_t = pool.tile([128, AOUT], f32)
            nc.vector.memset(out_t[:], 0.0)

            # ---- gathers: reg first (feeds extraction + math) ----
            gt_reg = pool.tile([128, 2 * Jr, 64], f32)
            nc.gpsimd.dma_gather(
                out_ap=gt_reg[:],
                in_ap=regT.ap().rearrange("(r e) -> r e", e=64),
                idxs_ap=idx_t[:, 0:KR // 16],
                num_idxs=KR, num_idxs_reg=KR, elem_size=64,
                single_packet=False, queue_num=0)
            gt_cls = pool.tile([128, J2 * 64 + 8], f32)
            nc.gpsimd.dma_gather(
                out_ap=gt_cls[:, 0:2 * J0 * 64].rearrange(
                    "p (a b) -> p a b", b=64),
                in_ap=clsT.ap()[0:80 * HW].rearrange("(r e) -> r e", e=64),
                idxs_ap=idx_t[:, KR // 16:(KR + K0) // 16],
                num_idxs=K0, num_idxs_reg=K0, elem_size=64,
                single_packet=False, queue_num=1)
            nc.gpsimd.dma_gather(
                out_ap=gt_cls[:, 2 * J0 * 64:J2 * 64].rearrange(
                    "p (a b) -> p a b", b=64),
                in_ap=clsT.ap()[80 * HW:].rearrange("(r e) -> r e", e=64),
                idxs_ap=idx_t[:, (KR + K0) // 16:(KR + K0 + K1) // 16],
                num_idxs=K1, num_idxs_reg=K1, elem_size=64,
                single_packet=False, queue_num=2)

            tt = nc.vector.tensor_tensor
            ts = nc.vector.tensor_scalar
            stt = nc.vector.scalar_tensor_tensor
            act = nc.scalar.activation

            def T(shape, tag):
                return pool.tile([128] + shape, f32, name=tag, tag=tag)

            def hfv(i, n=1):
                if n == 1:
                    return hf[:, i, :]
                return hf[:, i:i + n, :]

            # ---- reg extraction: 4-wide one-hot per (cell, win) ----
            ext = T([4, Jr, 5], "ext")
            grap = gt_reg[:].rearrange("p a b -> p (a b)")
            ohv = _V(big[:, OH4:OH4 + Jr * 4], [[4, Jr], [0, 5], [1, 4]])
            for cw in range(4):
                cell, win = cw >> 1, cw & 1
                gv = _V(grap, [[128, Jr], [1, 5], [16, 4]],
                        extra_off=cell * 16 + win * 8)
                prod = T([Jr, 5, 4], f"prodr{cw}")
                tt(out=prod[:], in0=gv, in1=ohv, op=Alu.mult)
                nc.vector.tensor_reduce(out=ext[:, cw], in_=prod[:],
                                        axis=X, op=Alu.add)

            eap = ext[:].rearrange("p a b c -> p (a b c)")
            pobj_v = _V(eap, [[5 * Jr, 4], [5, Jr]], extra_off=0)
            pr01_v = _V(eap, [[1, 2], [5 * Jr, 4], [5, Jr]], extra_off=1)
            pr23_v = _V(eap, [[1, 2], [5 * Jr, 4], [5, Jr]], extra_off=3)

            # pobj for phase B (fills the tanh/sigmoid latency)
            nc.vector.tensor_copy(
                out=out_t[:, 2 * G2:3 * G2].rearrange(
                    "p (a b) -> p a b", b=Jr),
                in_=pobj_v)

            def r4(apx):   # [128, 2, G2] -> [128, 2, 4, Jr]
                return apx.rearrange("p c (a b) -> p c a b", b=Jr)

            # ---- SIoU math (manually scheduled for ACT overlap) ----
            t01 = T([2, G2], "t01")
            act(r4(t01[:]), pr01_v, Act.Tanh)
            sg = T([2, G2], "sg")
            act(r4(sg[:]), pr23_v, Act.Sigmoid)

            txy = T([2, G2], "txy")
            tt(out=txy[:], in0=t01[:], in1=hfv(F_GIJ, 2), op=Alu.add)
            b1a = T([2, G2], "b1a")
            stt(out=b1a[:], in0=sg[:], scalar=-80.0, in1=txy[:],
                op0=Alu.mult, op1=Alu.add)
            b1b = T([2, G2], "b1b")
            stt(out=b1b[:], in0=sg[:], scalar=80.0, in1=txy[:],
                op0=Alu.mult, op1=Alu.add)
            wh1 = T([2, G2], "wh1")
            tt(out=wh1[:], in0=b1b[:], in1=b1a[:], op=Alu.subtract)
            s2 = T([2, G2], "s2")
            tt(out=s2[:], in0=hfv(F_SXY, 2), in1=b1a[:], op=Alu.subtract)
            tt(out=s2[:], in0=s2[:], in1=b1b[:], op=Alu.subtract)

            b2a = hfv(F_B2A, 2)
            b2b = hfv(F_B2B, 2)
            mn = T([2, G2], "mn")
            tt(out=mn[:], in0=b1b[:], in1=b2b, op=Alu.min)
            mx = T([2, G2], "mx")
            tt(out=mx[:], in0=b1a[:], in1=b2a, op=Alu.max)
            dcl = T([2, G2], "dcl")
            tt(out=dcl[:], in0=mn[:], in1=mx[:], op=Alu.subtract)
            ts(dcl[:], dcl[:], 0.0, None, Alu.max)
            inter = T([G2], "inter")
            tt(out=inter[:], in0=dcl[:, 0, :], in1=dcl[:, 1, :], op=Alu.mult)
            area1 = T([G2], "area1")
            tt(out=area1[:], in0=wh1[:, 0, :], in1=wh1[:, 1, :], op=Alu.mult)
            u = T([G2], "u")
            stt(out=u[:], in0=inter[:], scalar=-1.0, in1=area1[:],
                op0=Alu.mult, op1=Alu.add)
            tt(out=u[:], in0=u[:], in1=hfv(F_AREA2), op=Alu.add)
            invu = T([G2], "invu")
            nc.vector.reciprocal(invu[:], u[:])
            iou0 = T([G2], "iou0")
            tt(out=iou0[:], in0=inter[:], in1=invu[:], op=Alu.mult)

            mx2 = T([2, G2], "mx2")
            tt(out=mx2[:], in0=b1b[:], in1=b2b, op=Alu.max)
            mn2 = T([2, G2], "mn2")
            tt(out=mn2[:], in0=b1a[:], in1=b2a, op=Alu.min)
            cwh = T([2, G2], "cwh")
            tt(out=cwh[:], in0=mx2[:], in1=mn2[:], op=Alu.subtract)
            invcw = T([2, G2], "invcw")
            nc.vector.reciprocal(invcw[:], cwh[:])
            rr0 = T([2, G2], "rr0")
            tt(out=rr0[:], in0=s2[:], in1=invcw[:], op=Alu.mult)
            gr = T([2, G2], "gr")
            tt(out=gr[:], in0=rr0[:], in1=rr0[:], op=Alu.mult)

            wh2t = hfv(F_WH2, 2)
            dwh = T([2, G2], "dwh")
            tt(out=dwh[:], in0=wh1[:], in1=wh2t, op=Alu.subtract)
            adwh = T([2, G2], "adwh")
            stt(out=adwh[:], in0=dwh[:], scalar=-1.0, in1=dwh[:],
                op0=Alu.mult, op1=Alu.max)
            mxw = T([2, G2], "mxw")
            tt(out=mxw[:], in0=wh1[:], in1=wh2t, op=Alu.max)
            nc.vector.reciprocal(mxw[:], mxw[:])
            omw = T([2, G2], "omw")
            tt(out=omw[:], in0=adwh[:], in1=mxw[:], op=Alu.mult)
            ewh = T([2, G2], "ewh")
            act(ewh[:], omw[:], Act.Exp, scale=-1.0)

            # angle cost: 2*sin1*sin2 = 2*|s2x*s2y|/ssum  (sin1^2+sin2^2=1)
            sqd = T([2, G2], "sqd")
            tt(out=sqd[:], in0=s2[:], in1=s2[:], op=Alu.mult)
            ssum = T([G2], "ssum")
            tt(out=ssum[:], in0=sqd[:, 0, :], in1=sqd[:, 1, :], op=Alu.add)
            rs = T([G2], "rs")
            nc.vector.reciprocal(rs[:], ssum[:])
            pxy = T([G2], "pxy")
            tt(out=pxy[:], in0=s2[:, 0, :], in1=s2[:, 1, :], op=Alu.mult)
            apxy = T([G2], "apxy")
            stt(out=apxy[:], in0=pxy[:], scalar=-1.0, in1=pxy[:],
                op0=Alu.mult, op1=Alu.max)
            gam4 = T([G2], "gam4")
            tt(out=gam4[:], in0=apxy[:], in1=rs[:], op=Alu.mult)
            ts(gam4[:], gam4[:], 0.5, -0.5, Alu.mult, Alu.add)
            tt(out=gr[:], in0=gr[:], in1=_V(gam4[:], [[0, 2], [1, G2]]),
               op=Alu.mult)
            eg = T([2, G2], "eg")
            act(eg[:], gr[:], Act.Exp)

            oe = T([2, G2], "oe")
            ts(oe[:], ewh[:], -1.0, 1.0, Alu.mult, Alu.add)
            tt(out=oe[:], in0=oe[:], in1=oe[:], op=Alu.mult)
            tt(out=oe[:], in0=oe[:], in1=oe[:], op=Alu.mult)
            shp = T([G2], "shp")
            tt(out=shp[:], in0=oe[:, 0, :], in1=oe[:, 1, :], op=Alu.add)

            t_eg = T([G2], "t_eg")
            tt(out=t_eg[:], in0=eg[:, 0, :], in1=eg[:, 1, :], op=Alu.add)
            c1 = T([G2], "c1")
            stt(out=c1[:], in0=shp[:], scalar=-1.0, in1=t_eg[:],
                op0=Alu.mult, op1=Alu.add)
            ts(c1[:], c1[:], 0.5, -1.0, Alu.mult, Alu.add)
            iou_v = out_t[:, 0:G2]
            tt(out=iou_v, in0=iou0[:], in1=c1[:], op=Alu.add)

            # sum(iou*m) partial per partition
            scr = T([G2], "scr")
            stt(out=scr[:], in0=iou_v, scalar=1.0, in1=hfv(F_M),
                op0=Alu.mult, op1=Alu.mult,
                accum_out=out_t[:, 3 * G2:3 * G2 + 1])

            # ---- class extraction: two-level one-hot (hi 8 x lo 8) ----
            strip = T([J2, 9], "strip")
            prod1 = T([J2, 9, 8], "prod1")
            gv1 = _V(gt_cls[:], [[64, J2], [1, 9], [8, 8]])
            ohhiv = _V(big[:, OHHI:OHHI + J2 * 8], [[8, J2], [0, 9], [1, 8]])
            tt(out=prod1[:], in0=gv1, in1=ohhiv, op=Alu.mult)
            nc.vector.tensor_reduce(out=strip[:], in_=prod1[:],
                                    axis=X, op=Alu.add)
            ohlov = big[:, OHLO:OHLO + J2 * 8].rearrange(
                "p (a b) -> p a b", b=8)
            pg = T([2, J2], "pg")
            prod2 = T([J2, 8], "prod2")
            tt(out=prod2[:], in0=strip[:, :, 0:8], in1=ohlov, op=Alu.mult)
            nc.vector.tensor_reduce(out=pg[:, 0], in_=prod2[:],
                                    axis=X, op=Alu.add)
            prod3 = T([J2, 8], "prod3")
            tt(out=prod3[:], in0=_V(strip[:].rearrange("p a b -> p (a b)"),
                                    [[9, J2], [1, 8]], extra_off=1),
               in1=ohlov, op=Alu.mult)
            nc.vector.tensor_reduce(out=pg[:, 1], in_=prod3[:],
                                    axis=X, op=Alu.add)
            ts(pg[:], pg[:], 1e-38, None, Alu.max)
            lnt = T([2, J2], "lnt")
            act(lnt[:], pg[:], Act.Ln)
            lnp_in = _V(lnt[:].rearrange("p a b -> p (a b)"),
                        [[J2, 2], [1, 2], [2, Jr]])
            tt(out=out_t[:, G2:2 * G2].rearrange(
                   "p (c w j) -> p c w j", c=2, w=2),
               in0=lnp_in,
               in1=hfv(F_MCLSV).rearrange("p (c w j) -> p c w j", c=2, w=2),
               op=Alu.mult)

            nc.sync.dma_start(out=aoutT.ap(), in_=out_t[:])

    lower_extended_insts(nc)
    _split_excess_waits(nc)
    return nc


# ---------------- phase B program ----------------

def _build_phase_b(meta):
    G2 = meta["G2"]
    AOUT = 3 * G2 + 8
    # merged input: [aout(iou host-masked) | fv/2 | imean | pobjd]
    BINW = AOUT + G2 + 1 + 400

    nc = bass.Bass("TRN2", debug=False)
    binT = nc.dram_tensor("binall", [128, BINW], f32, kind="ExternalInput")
    boutT = nc.dram_tensor("bout", [128, 8], f32, kind="ExternalOutput")

    with TileContext(nc) as tc:
        with tc.tile_pool(name="sbuf", bufs=1) as pool:
            bi = pool.tile([128, BINW], f32)
            nc.sync.dma_start(out=bi[:], in_=binT.ap())

            ob = pool.tile([128, 8], f32)
            nc.vector.memset(ob[:], 0.0)

            iou_v = bi[:, 0:G2]
            lnp_v = bi[:, G2:2 * G2]
            pox = bi[:, 2 * G2:3 * G2]
            fv = bi[:, AOUT:AOUT + G2]
            im = bi[:, AOUT + G2:AOUT + G2 + 1]
            pod = bi[:, AOUT + G2 + 1:BINW]

            tt = nc.vector.tensor_tensor
            ts = nc.vector.tensor_scalar
            stt = nc.vector.scalar_tensor_tensor

            def T(shape, tag):
                return pool.tile([128] + shape, f32, name=tag, tag=tag)

            f_v = T([G2], "f")
            tt(out=f_v[:], in0=iou_v,
               in1=im.to_broadcast([128, G2]), op=Alu.is_gt)

            # bout0 = sum f*iou ; bout1 = sum f*lnp ; bout2 = sum f
            s0 = T([G2], "s0")
            stt(out=s0[:], in0=iou_v, scalar=1.0, in1=f_v[:],
                op0=Alu.mult, op1=Alu.mult, accum_out=ob[:, 0:1])
            s1 = T([G2], "s1")
            stt(out=s1[:], in0=lnp_v, scalar=1.0, in1=f_v[:],
                op0=Alu.mult, op1=Alu.mult, accum_out=ob[:, 1:2])
            nc.vector.tensor_reduce(out=ob[:, 2:3], in_=f_v[:],
                                    axis=X, op=Alu.add)

            # obj corr: f*(sl1(pobj-iou)*fval - 0.375*pobj^2)
            # sl1 = 0.5*mm*(2*ad - mm), mm = min(ad,1); the 0.5 is folded
            # into fv (host ships fval/2)
            d = T([G2], "d")
            tt(out=d[:], in0=pox, in1=iou_v, op=Alu.subtract)
            ad = T([G2], "ad")
            stt(out=ad[:], in0=d[:], scalar=-1.0, in1=d[:],
                op0=Alu.mult, op1=Alu.max)
            mm_ = T([G2], "mm_")
            ts(mm_[:], ad[:], 1.0, None, Alu.min)
            t2 = T([G2], "t2")
            stt(out=t2[:], in0=ad[:], scalar=2.0, in1=mm_[:],
                op0=Alu.mult, op1=Alu.subtract)
            tt(out=t2[:], in0=t2[:], in1=mm_[:], op=Alu.mult)
            tt(out=t2[:], in0=t2[:], in1=fv, op=Alu.mult)
            po2 = T([G2], "po2")
            stt(out=po2[:], in0=pox, scalar=-0.375, in1=pox,
                op0=Alu.mult, op1=Alu.mult)
            tt(out=t2[:], in0=t2[:], in1=po2[:], op=Alu.add)
            s4 = T([G2], "s4")
            stt(out=s4[:], in0=t2[:], scalar=1.0, in1=f_v[:],
                op0=Alu.mult, op1=Alu.mult, accum_out=ob[:, 3:4])

            # dense obj baseline partial: sum(pobj^2)
            s5 = pool.tile([128, 400], f32)
            stt(out=s5[:], in0=pod, scalar=1.0, in1=pod,
                op0=Alu.mult, op1=Alu.mult, accum_out=ob[:, 4:5])

            nc.sync.dma_start(out=boutT.ap(), in_=ob[:])

    lower_extended_insts(nc)
    _split_excess_waits(nc)
    return nc


# ---------------- host-side patches ----------------

def _sl1(x):
    ax = abs(x)
    return 0.5 * x * x if ax < 1.0 else ax - 0.5


def _host_patches(per_core, meta, aouts, iou_mean):
    """Returns (corr_patch, s2_patch): corr_patch is subtracted from the
    device obj-corr sum (collision losers); s2_patch is added to the
    device sum f*lnp (class phi=63 crossings)."""
    G2 = meta["G2"]
    corr_patch = 0.0
    s2_patch = 0.0
    for k, d in enumerate(per_core):
        a = aouts[k]
        iou = a[:, 0:G2]
        pobj = a[:, 2 * G2:3 * G2]
        hb = d["hostb"].reshape(128, 3, G2)
        m = hb[:, 0, :]
        mkB = hb[:, 2, :]
        f = (iou > iou_mean) & (m > 0)

        # nperb for this core's two batches (exact integer counts)
        npA = max(float((f & (mkB < 0.5)).sum()), 0.5)
        npB = max(float((f & (mkB > 0.5)).sum()), 0.5)
        fvalA = 6400.0 / npA
        fvalB = 6400.0 / npB

        # collision dedup: group f-positive candidates by cell id
        cells = d["candcell"]
        fpos = f & (cells >= 0)
        if fpos.any():
            cid = cells[fpos]
            orig = d["candorig"][fpos]
            iouv = iou[fpos]
            pov = pobj[fpos]
            isB = mkB[fpos] > 0.5
            order = np.argsort(cid, kind="stable")
            cid, orig, iouv, pov, isB = (cid[order], orig[order],
                                         iouv[order], pov[order], isB[order])
            i = 0
            n = len(cid)
            while i < n:
                jx = i
                while jx + 1 < n and cid[jx + 1] == cid[i]:
                    jx += 1
                if jx > i:
                    widx = i + int(np.argmax(orig[i:jx + 1]))
                    for t in range(i, jx + 1):
                        if t == widx:
                            continue
                        fval = fvalB if isB[t] else fvalA
                        corr_patch += (_sl1(float(pov[t]) - float(iouv[t]))
                                       * fval - 0.375 * float(pov[t]) ** 2)
                i = jx + 1

        # class crossing patch
        for (p, col, pv) in d["crossing"]:
            if f[p, col]:
                s2_patch += math.log(max(pv, 1e-38))
    return corr_patch, s2_patch


# ---------------- main entry ----------------

_CACHE = {}


def kernel(preds, targets):
    per_core, meta = _prep(preds, targets)

    key = (meta["J0"], meta["J1"])
    if key not in _CACHE:
        _CACHE[key] = (_build_phase_a(meta), _build_phase_b(meta))
    nc_a, nc_b = _CACHE[key]

    core_ids = list(range(NCORES))
    in_maps_a = [dict(regarr=d["regarr"], clsarr=d["clsarr"],
                      idx16=d["idx16"], big=d["big"]) for d in per_core]
    res_a = run_bass_kernel_spmd(nc_a, in_maps_a, core_ids)

    G2 = meta["G2"]
    aouts = [res_a.results[k]["aout"] for k in core_ids]
    sum_im = sum(float(a[:, 3 * G2].sum(dtype=np.float64)) for a in aouts)
    iou_mean = np.float32(np.float32(sum_im) / np.float32(meta["cnt_m"]))

    imean_arr = np.full((128, 1), iou_mean, np.float32)
    in_maps_b = []
    for k in core_ids:
        d = per_core[k]
        a = aouts[k]
        hb = d["hostb"]
        m_h, mkA, mkB = hb[:, 0], hb[:, 1], hb[:, 2]
        fh = (a[:, 0:G2] > iou_mean) & (m_h > 0)
        npA = max(float((fh & (mkA > 0.5)).sum()), 0.5)
        npB = max(float((fh & (mkB > 0.5)).sum()), 0.5)
        fv2 = (mkA * np.float32(3200.0 / npA)
               + mkB * np.float32(3200.0 / npB)).astype(np.float32)
        d["fvals"] = (6400.0 / npA, 6400.0 / npB)
        am = a.copy()
        am[:, 0:G2] = np.where(m_h > 0, a[:, 0:G2], np.float32(-1e4))
        binall = np.concatenate(
            [am, fv2, imean_arr, d["pobjd"]], axis=1).astype(np.float32)
        in_maps_b.append(dict(binall=binall))
    res_b = run_bass_kernel_spmd(nc_b, in_maps_b, core_ids)

    bouts = [res_b.results[k]["bout"] for k in core_ids]
    Sfi = sum(float(o[:, 0].sum(dtype=np.float64)) for o in bouts)
    S2 = sum(float(o[:, 1].sum(dtype=np.float64)) for o in bouts)
    cnt_f = max(sum(float(o[:, 2].sum(dtype=np.float64)) for o in bouts), 1.0)
    corr = sum(float(o[:, 3].sum(dtype=np.float64)) for o in bouts)
    base = sum(float(o[:, 4].sum(dtype=np.float64)) for o in bouts)

    corr_patch, s2_patch = _host_patches(per_core, meta, aouts, iou_mean)

    iou_loss = np.float32((cnt_f - Sfi) / cnt_f)
    cls_loss = np.float32(-(S2 + s2_patch) / cnt_f)
    obj_loss = np.float32((0.375 * base + corr - corr_patch) / (N * HW))
    loss = np.float32(iou_loss * 8 + obj_loss * 16 + cls_loss)
    return (iou_loss, obj_loss, cls_loss, loss)
